# revision 28
# baseline (speedup 1.0000x reference)
"""Trainium2 Bass kernel for DeformCapsNet (conv backbone + 2 capsule layers with
dynamic routing + SE gating). Data-parallel over batch: 1 sample per NeuronCore.

Self-contained: hardcodes all shapes; host-side numpy only rearranges weights /
im2cols the 3-channel input; all FLOPs run on device.

Algorithm notes (validated in numpy against the jax reference):
  - routing logits are constant along the atom axis -> kept as [p, o*c].
  - 3x3 convs = 9 shifted matmuls over a zero-padded 98x98 flat grid (9604
    positions, padded to 9728 = 76 tiles of 128); border positions compute
    garbage votes that are masked to zero before any reduction over p.
  - votes conv emitted in [p, o*c*a] layout (lhsT = shifted f tile, rhs = W);
    fp16 operands, fp32 PSUM. The conv BIAS matmuls are dropped: inst_b/cls_b
    are zeros by the problem spec (fill: zeros); kernel() asserts this.
  - votes ship to HBM in fp8e4m3 as per-tile [v | vT] groups ([128, 2048]
    each, one DMA per tile; vT via 8 PE transposes of the fp8 v). Numpy
    simulation of the full pipeline shows fp8 vote storage & fp8 routes add
    no error over the baseline (9.9e-3 vs 1.01e-2): the error budget is
    dominated by the fp8 activation quantization in the agree path.
  - iteration-1 preact (uniform 1/8 route) = (1/8) sum_p votes: folded into
    9 small matmuls on pre-summed f: sum_p v[p,:] = sum_d (sum_{p in win_d}
    f[p]) @ W_d. The f window sums are separable row/col DVE reductions
    computed ONCE (f is shared by both capsule layers). Replaces 76 per-tile
    preact matmuls per layer with 18.
  - routing iterations 2/3: agree via fp8 matmuls agT[oc,p] = sum_s
    Mblk_s.T @ vT_s with DoubleRow perf mode (slice pairs -> K=256 per
    instruction); preact via fp8 DoubleRow matmuls pairing consecutive
    p-tiles (r8 softmax outputs written as fp8 pair buffers). PSUM fp32.
  - agree back-transpose in fp16 (agT staged to fp16 SBUF).
  - routing phases run a software pipeline (agree(t) | mid(t-2) |
    preact-pair at odd t) so the in-order PE never stalls on the softmax
    chain; phase B interleaves layer-1's conv with layer-0 iteration 2.
  - DMA-written vote-group ring buffers live at manually placed high-SBUF
    addresses (RING_BASE up): the liveness packer's WAR sync for DMA writes
    into reused ranges was observed to be under-counted; same-tensor ring
    reuse is soundly synced. build_program() asserts no pooled allocation
    enters the ring region.
  - measured end-to-end error vs the fp32 reference ~1e-2 (gate 2e-2).
"""

import numpy as np
from contextlib import ExitStack

import concourse.bass as bass
import concourse.bacc as bacc
import concourse.mybir as mybir
import concourse.tile as tile
from concourse.bass_utils import run_bass_kernel_spmd
from concourse.masks import make_identity

N_CORES = 8
PH = PW = 98
NP = PH * PW            # 9604 padded grid positions
NT = 76                 # p-tiles of 128
NPT = NT * 128          # 9728
G = 128                 # guard columns on each side of f buffers
RING_BASE = 139264      # manual high-SBUF region for DMA-written ring buffers
OFFS = [(dy, dx) for dy in (-1, 0, 1) for dx in (-1, 0, 1)]
DELTAS = [dy * PW + dx for dy, dx in OFFS]
F32 = mybir.dt.float32
F16 = mybir.dt.float16
F8 = mybir.dt.float8e4
AF = mybir.ActivationFunctionType
ALU = mybir.AluOpType
DR = mybir.MatmulPerfMode.DoubleRow


def _gmask():
    """[128, 512] f16: 1.0 at (g*16+a, 64s + 8s+g) for the Mblk spread.
    64-wide slices: the DoubleRow lhsT k-tile stride must be 16B-aligned."""
    m = np.zeros((128, 512), dtype=np.float16)
    for g in range(8):
        for a in range(16):
            for s in range(8):
                m[g * 16 + a, 64 * s + 8 * s + g] = 1.0
    return m


def _lspread():
    """[16, 128] f16: L[a, p] = 1 iff p % 16 == a (atom selector for act_big)."""
    m = np.zeros((16, 128), dtype=np.float16)
    for p in range(128):
        m[p % 16, p] = 1.0
    return m


def _diag_mask512():
    """[64, 512] mask: 1.0 at col (oc%32)*16+a for row oc."""
    m = np.zeros((64, 512), dtype=np.float32)
    for oc in range(64):
        m[oc, (oc % 32) * 16:(oc % 32) * 16 + 16] = 1.0
    return m


def _border_mask():
    """1.0 at interior padded-flat positions, 0.0 at borders/tail. [128, NT]."""
    m = np.zeros((PH, PW), dtype=np.float32)
    m[1:-1, 1:-1] = 1.0
    flat = np.zeros(NPT, dtype=np.float32)
    flat[:NP] = m.reshape(-1)
    return flat.reshape(NT, 128).T.copy()  # [p_local, t]


def _zero_f_borders(nc, buf, nparts):
    """memset border cols of an [nparts, G+NPT+G] padded f buffer (interior part)."""
    v = buf[:nparts, :]
    nc.vector.memset(v[:, G:G + PW], 0.0)                       # top row
    nc.vector.memset(v[:, G + NP - PW:G + NPT], 0.0)            # bottom row + tail
    lc = v[:, G:G + NP].rearrange("p (r c) -> p r c", c=PW)
    nc.vector.memset(lc[:, :, 0:1], 0.0)                        # left col
    nc.vector.memset(lc[:, :, PW - 1:PW], 0.0)                  # right col


def build_program():
    nc = bacc.Bacc(
        "TRN2", target_bir_lowering=False, debug=False, num_devices=N_CORES
    )

    def inp(name, shape, dt=F16):
        return nc.dram_tensor(name, shape, dt, kind="ExternalInput").ap()

    io = {
        "xdup": inp("xdup", [27, NPT]),
        "w1": inp("w1", [27, 64]),
        "b1": inp("b1", [64, 1], F32),
        "w2": inp("w2", [64, 9 * 128]),
        "b2": inp("b2", [128, 1], F32),
        "wcaps": inp("wcaps", [2, 128, 9, 1024]),
        "bias2": inp("bias2", [64, 2, 16], F32),
        "vmask": inp("vmask", [128, NT], F32),
        "dmask": inp("dmask", [64, 512], F32),
        "se1": inp("se1", [33, 4], F32),
        "se2": inp("se2", [5, 32], F32),
        "out": nc.dram_tensor("out", [64, 32], F32, kind="ExternalOutput").ap(),
        # per-tile-pair vote groups: [v(t0) | vT(t0) | v(t1) | vT(t1)], fp8
        "votes": nc.dram_tensor("votes_scratch", [2, NT // 2, 128, 4096],
                                F8).ap(),
        "gmask": inp("gmask", [128, 512]),
        "lspread": inp("lspread", [16, 128]),
    }

    with tile.TileContext(nc) as tc, ExitStack() as ctx:
        _body(ctx, tc, io)
    nc.compile()
    for alloc in nc.m.functions[0].allocations:
        for ml in getattr(alloc, "memorylocations", None) or []:
            if (getattr(ml, "allocated", False) and ml.addr is not None
                    and getattr(ml, "type", "") == "SB"
                    and not ml.name.startswith(("vgr", "xdup_sb"))
                    and len(ml.dims) >= 2):
                assert ml.addr + ml.dims[1] <= RING_BASE, \
                    f"pool alloc {ml.name} overlaps ring region"
    return nc


class _State:
    pass


def _finish_iter(nc, st, l, pre_ps, act_out, act16_out):
    """pre_ps [64,512] psum -> diag-extract + bias -> squash -> act [64,16]."""
    pool = st.small
    b2l = st.bias2_sb[:, l * 16:(l + 1) * 16]
    pre = pool.tile([64, 16], F32, tag="pre")
    for h in range(2):
        rows = slice(32 * h, 32 * h + 32)
        masked = pool.tile([32, 512], F32, tag=f"maskd{h}", bufs=1,
                           name=f"maskd{h}")
        nc.vector.tensor_mul(masked, pre_ps[h][rows, :], st.dmask_sb[rows, :])
        nc.vector.reduce_sum(pre[rows, :],
                             masked.rearrange("p (g a) -> p a g", a=16),
                             axis=mybir.AxisListType.X)
    nc.vector.tensor_add(pre, pre, b2l)
    sq = pool.tile([64, 16], F32, tag="sq")
    ssum = pool.tile([64, 1], F32, tag="ssum")
    nc.scalar.activation(sq, pre, AF.Square, accum_out=ssum)
    nrm = pool.tile([64, 1], F32, tag="nrm")
    nc.scalar.activation(nrm, ssum, AF.Sqrt)
    den = pool.tile([64, 1], F32, tag="den")
    nc.vector.tensor_scalar_add(den, ssum, 1.0)
    rec = pool.tile([64, 1], F32, tag="rec")
    nc.vector.reciprocal(rec, den)
    scl = pool.tile([64, 1], F32, tag="scl")
    nc.vector.tensor_mul(scl, nrm, rec)
    if act_out is not None:
        nc.vector.tensor_scalar(act_out, pre, scl, None, op0=ALU.mult)
    if act16_out is not None:
        nc.vector.tensor_scalar(act16_out, pre, scl, None, op0=ALU.mult)


def _pre_bank(nc, pool, tag):
    """Fresh pair of [64,512] preact psum banks (one per 512-col vote half:
    DoubleRow matmul dst must start at partition 0, so the halves can't be
    packed into quadrants of one bank), explicitly zeroed by DVE. All preact
    matmuls then use start=False: they accumulate where has_written bits are
    stale-set (onto the zeros) and overwrite-zero where clear — correct either
    way, and independent of whole-bank bit-clear semantics."""
    pre = []
    for h in range(2):
        p = pool.tile([64, 512], mybir.dt.float32, tag=f"{tag}{h}", bufs=1,
                      name=f"{tag}{h}")
        nc.vector.memset(p, 0.0)
        pre.append(p)
    return pre


def _fsums_compute(nc, st):
    """Pre-summed f for the uniform-route iteration-1 preact:
    fsums[k, d] = (1/8) * sum over the 96x96 window shifted by delta d of
    f[k, .]. Separable: 3 column-window row sums, then 9 row-window sums.
    Writes st.fsrep [128, 9*64] f16 (each delta's column replicated 64x so
    M=32 slices of it can be fed to the PE as the stationary operand)."""
    fi = st.f_buf[:, G:G + NP].rearrange("k (r c) -> k r c", c=PW)
    rs = [st.small.tile([128, PH], F32, tag=f"rs{i}", bufs=1, name=f"rs{i}")
          for i in range(3)]
    for i, dx in enumerate((-1, 0, 1)):
        nc.vector.reduce_sum(rs[i], fi[:, :, 1 + dx:97 + dx],
                             axis=mybir.AxisListType.X)
    fs32 = st.small.tile([128, 9], F32, tag="fs32", bufs=1)
    for d, (dy, dx) in enumerate(OFFS):
        nc.vector.reduce_sum(fs32[:, d:d + 1], rs[dx + 1][:, 1 + dy:97 + dy],
                             axis=mybir.AxisListType.X)
    fs16 = st.small.tile([128, 9], F16, tag="fs16", bufs=1)
    nc.vector.tensor_scalar_mul(fs16, fs32, 0.125)
    nc.vector.tensor_copy(
        st.fsrep.rearrange("p (d m) -> p d m", m=64),
        fs16.unsqueeze(2).broadcast_to((128, 9, 64)))


def _iter1_preact(nc, st, pre_ps, wl):
    """preact_1 = (1/8) sum_p votes, as 18 fsums x W matmuls (fp16, full
    M=64 — every output row carries the same delta sum; the diag extract in
    _finish_iter keeps the valid ones)."""
    for d in range(9):
        for h in range(2):
            nc.tensor.matmul(
                pre_ps[h],
                st.fsrep[:, d * 64:d * 64 + 64],
                wl[:, d * 1024 + 512 * h:d * 1024 + 512 * h + 512],
                start=False, stop=(d == 8), skip_group_check=True)


def _emit_conv_tile(nc, st, l, t, wl):
    """Votes conv for p-tile t of layer l: 18 fp16 matmuls (two [128,512]
    halves in separate PSUM banks), masked fp8 copy, 8 fp8 PE transposes.
    Tiles are assembled in PAIRS into one [128, 4096] group
    [v(t0) | v(t1) | vT slice-major (s, tile, p)] and shipped with a single
    DMA per pair (halves the Sync-queue store traffic; the slice-major vT
    lets the agree matmuls stream both tiles in one instruction)."""
    tt = t % 2
    if tt == 0:
        st.cur_group = st.vvA_pool.tile([128, 4096], F8, tag="vv",
                                        name="group")
    group = st.cur_group
    base = G + t * 128
    vps_h = [st.ps_votes.tile([128, 512], F32, tag="vps", name=f"vps{h}")
             for h in range(2)]
    # delta-outer order: each shifted f-tile stationary operand feeds both
    # output halves back-to-back (the halves accumulate in different PSUM
    # banks so per-half starts are safe)
    for i, d in enumerate(DELTAS):
        for h in range(2):
            nc.tensor.matmul(vps_h[h],
                             st.f_buf[:, base + d:base + d + 128],
                             wl[:, i * 1024 + h * 512:i * 1024 + h * 512 + 512],
                             start=(i == 0), stop=(i == 8))
    # masked fp8 copy: half h as soon as its 9 matmuls finish
    for h in range(2):
        nc.scalar.activation(
            group[:, tt * 1024 + h * 512:tt * 1024 + h * 512 + 512],
            vps_h[h], AF.Copy, scale=st.vmask_sb[:, t:t + 1])
    # fp8 PE transpose requires output element step of 2: write into every
    # other fp8 slot of a double-width PSUM tile, compact on the DVE copy
    vT_ps = st.ps_vT.tile([128, 2048], F8, tag="vtp", bufs=1)
    vT_v = vT_ps.rearrange("p (s x two) -> p s x two", s=8, two=2)
    for s in range(8):
        nc.tensor.transpose(vT_v[:, s, :, 0],
                            group[:, tt * 1024 + s * 128:tt * 1024 + s * 128 + 128],
                            st.ident8)
    dst = group[:, 2048:4096].rearrange("p (s q x) -> p s q x", s=8, q=2)
    nc.vector.tensor_copy(dst[:, :, tt, :], vT_v[:, :, :, 0])
    if tt == 1:
        nc.sync.dma_start(st.votes_d[l, t // 2], group)


def _softmax_r8(nc, st, lsl, r8_dst):
    """logits slice [128,128] (one group = 2 tiles) -> fp8 routes
    (softmax over capsules, both tiles in one op chain)."""
    e_t = st.rt_pool.tile([128, 128], F32, tag="et")
    nc.scalar.activation(e_t, lsl, AF.Exp)
    s_t = st.rt_pool.tile([128, 16], F32, tag="st")
    nc.vector.reduce_sum(s_t, e_t.rearrange("p (o c) -> p o c", c=8),
                         axis=mybir.AxisListType.X)
    rc = st.rt_pool.tile([128, 16], F32, tag="rc")
    nc.vector.reciprocal(rc, s_t)
    nc.vector.tensor_mul(
        r8_dst.rearrange("p (o c) -> p o c", c=8),
        e_t.rearrange("p (o c) -> p o c", c=8),
        rc.unsqueeze(2).broadcast_to((128, 16, 8)))


def _load_group(nc, st, l, j):
    """Load the [v|vT] group j (tiles 2j, 2j+1) of layer l into a ring slot.
    Alternates the issuing engine so the loads spread over two DMA queues
    (a single queue saturates below the per-core HBM peak). Both queues are
    ordered after the backbone's xdup reads (sync: behind phase A's stores;
    scalar: behind phase A's PSUM->SBUF copies), keeping the xdup/ring
    overlay safe."""
    g = st.vg_ring[st.ring_idx % len(st.vg_ring)]
    eng = nc.sync if st.ring_idx % 2 == 0 else nc.scalar
    st.ring_idx += 1
    eng.dma_start(g, st.votes_d[l, j])
    return g


def _agree_mms(nc, st, si, mblk, g, pagt):
    """4 DoubleRow agree matmuls per GROUP (vT slice pairs x both tiles,
    N=256), then the PSUM->SBUF fp16 copy so the bank frees early and
    backT's input is staged."""
    agT_ps = pagt.tile([64, 256], F32, tag="agt", bufs=1, name="agt")
    mpair = mblk.rearrange("p (s q) -> p s q", q=64)
    for sp in range(4):
        vt = g[:, 2048 + sp * 512:2048 + sp * 512 + 512] \
            .rearrange("p (two n) -> p two n", two=2)
        nc.tensor.matmul(agT_ps, mpair[:, 2 * sp:2 * sp + 2, :], vt,
                         start=(sp == 0), stop=(sp == 3), perf_mode=DR)
    agT_sb = st.agT_pool.tile([64, 256], F16, tag=f"agts{si}", bufs=3,
                              name=f"agts{si}")
    nc.scalar.activation(agT_sb, agT_ps, AF.Copy)
    return agT_sb


def _route_mid(nc, st, it, j, agT_sb, r8_dst, pagr):
    """agree back-transposes (fp16, both tiles of the group) + logits
    update + batched softmax -> fp8 route pair."""
    agr_ps = pagr.tile([128, 128], F16, tag="agrp", bufs=1, name="agrp")
    for tt in range(2):
        nc.tensor.transpose(agr_ps[:, tt * 64:tt * 64 + 64],
                            agT_sb[:, tt * 128:tt * 128 + 128],
                            st.ident16[:64, :64])
    lsl = st.logits[:, j * 128:(j + 1) * 128]
    if it == 2:
        nc.vector.tensor_copy(lsl, agr_ps)
    else:
        nc.vector.tensor_add(lsl, lsl, agr_ps)
    _softmax_r8(nc, st, lsl, r8_dst)


def _preact_pair(nc, st, pre_ps, r8p, g, last):
    """Two DoubleRow matmuls: preact += r8(t0).T@v(t0) + r8(t1).T@v(t1)."""
    rpair = r8p.rearrange("p (two m) -> p two m", two=2)
    vv = g[:, 0:2048].rearrange("p (two x) -> p two x", two=2)
    for h in range(2):
        nc.tensor.matmul(
            pre_ps[h],
            rpair,
            vv[:, :, 512 * h:512 * h + 512],
            start=False, stop=last, skip_group_check=True, perf_mode=DR)


def _mblk_build(nc, st, act16, mblk):
    """Spread act16 [64,16] into the block-diagonal lhsT Mblk [128, 8*72]:
    slice s cols [72s, 72s+64), nonzeros at (g*16+a, slice-col 8s+g).
    Engines can't shift partitions, so the spread goes via PE: actT = act.T,
    act_big = Lspread.T @ actT, Mblk = Gmask * act_big (one DVE op).
    PSUM temporaries ride the existing agr/agT pool slots (tag reuse) to
    stay inside the 8-bank budget."""
    actT_ps = st.ps_agr.tile([16, 64], F16, tag="agrp", name="mbt", bufs=1)
    nc.tensor.transpose(actT_ps, act16, st.ident16[:64, :64])
    actT_sb = st.small.tile([16, 64], F16, tag="mbts")
    nc.vector.tensor_copy(actT_sb, actT_ps)
    big_ps = st.ps_agT.tile([128, 64], F32, tag="agt", name="mbb", bufs=1)
    nc.tensor.matmul(big_ps, st.lspread, actT_sb, start=True, stop=True)
    act_big = st.small.tile([128, 64], F16, tag="mbbig")
    nc.vector.tensor_copy(act_big, big_ps)
    nc.vector.tensor_mul(
        mblk.rearrange("p (s q) -> p s q", q=64),
        st.gmask_sb.rearrange("p (s q) -> p s q", q=64),
        act_big.unsqueeze(1).broadcast_to((128, 8, 64)))


def _body(ctx, tc, io):
    nc = tc.nc
    st = _State()
    # ALL SBUF pools are open for the whole body: no cross-phase address
    # reuse (a reuse-WAR race was observed with scoped pools). Only PSUM
    # pools are phase-scoped (8 banks force reuse).
    persist = ctx.enter_context(tc.tile_pool(name="persist", bufs=1))
    st.small = ctx.enter_context(tc.tile_pool(name="small", bufs=2))
    st.rt_pool = ctx.enter_context(tc.tile_pool(name="rt", bufs=5))
    st.agT_pool = ctx.enter_context(tc.tile_pool(name="agT", bufs=3))
    st.r8_pool = ctx.enter_context(tc.tile_pool(name="r8", bufs=6))
    wl_pool = ctx.enter_context(tc.tile_pool(name="wl", bufs=1))
    fb_pool = ctx.enter_context(tc.tile_pool(name="fb", bufs=1))
    st.vvA_pool = ctx.enter_context(tc.tile_pool(name="vvA", bufs=4))
    st.votes_d = io["votes"]

    caps_ctx = ExitStack()
    ps_preA = caps_ctx.enter_context(
        tc.tile_pool(name="pspA", bufs=1, space="PSUM"))

    # ---- persistent tensors / constants ----
    st.f_buf = fb_pool.tile([128, G + NPT + G], F16)
    st.logits = persist.tile([128, NT * 64], F32)
    w1_sb = persist.tile([27, 64], F16)
    b1_sb = persist.tile([64, 1], F32)
    w2_sb = persist.tile([64, 9 * 128], F16)
    b2_sb = persist.tile([128, 1], F32)
    st.bias2_sb = persist.tile([64, 2 * 16], F32)
    st.vmask_sb = persist.tile([128, NT], F32)
    st.dmask_sb = persist.tile([64, 512], F32)
    se1_sb = persist.tile([33, 4], F32)
    se2_sb = persist.tile([5, 32], F32)
    st.ident = persist.tile([128, 128], F32)
    st.ident16 = persist.tile([128, 128], F16)
    st.ident8 = persist.tile([128, 128], F8)
    st.gmask_sb = persist.tile([128, 512], F16)
    st.lspread = persist.tile([16, 128], F16)
    st.fsrep = persist.tile([128, 9 * 64], F16)
    st.mblk = [persist.tile([128, 512], F8, name=f"mblk{i}") for i in range(2)]
    comb = persist.tile([64, 32], F32)
    # DMA-written ring buffers live at MANUALLY-placed high-SBUF addresses,
    # outside the liveness packer's reach: Tile's WAR sync for DMA writes
    # into packer-reused ranges was observed to be under-counted (race), while
    # same-tensor reuse sync is sound. build_program() asserts pools stay
    # below RING_BASE.
    off = RING_BASE

    def ring_at(name, width, dt):
        nonlocal off
        h = nc.alloc_sbuf_tensor_at(name, [128, width], dt, offset=off)
        off += width * mybir.dt.size(dt)
        return h.ap()

    st.vg_ring = [ring_at(f"vgr{i}", 4096, F8) for i in range(18)]
    st.ring_idx = 0
    assert off <= 212992, f"ring region overflow: {off}"

    def load_wl(l):
        """One resident weight buffer, reloaded per layer (tag-cycled)."""
        wl = wl_pool.tile([128, 9 * 1024], F16, tag="wl", name=f"wl{l}")
        for i in range(9):
            nc.scalar.dma_start(wl[:, i * 1024:(i + 1) * 1024],
                                io["wcaps"][l, :, i])
        return wl

    for name, sb in [("w1", w1_sb), ("b1", b1_sb), ("w2", w2_sb),
                     ("b2", b2_sb)]:
        nc.sync.dma_start(sb, io[name])
    for name, sb in [("vmask", st.vmask_sb), ("dmask", st.dmask_sb),
                     ("se1", se1_sb), ("se2", se2_sb),
                     ("gmask", st.gmask_sb), ("lspread", st.lspread)]:
        nc.scalar.dma_start(sb, io[name])
    nc.scalar.dma_start(st.bias2_sb, io["bias2"].rearrange("p l a -> p (l a)"))
    make_identity(nc, st.ident)
    make_identity(nc, st.ident16)
    nc.vector.tensor_copy(st.ident8, st.ident16)
    nc.vector.memset(st.f_buf[:, 0:G], 0.0)
    nc.vector.memset(st.f_buf[:, G + NPT:], 0.0)

    # ---- backbone ----
    with tc.tile_pool(name="backbone", bufs=1) as bb, \
         tc.tile_pool(name="psb", bufs=2, space="PSUM") as psb:
        # xdup's lifetime (backbone conv1) strictly precedes the ring's
        # first DMA write (phase B route loads, which the Sync queue issues
        # after phase A's stores) -> safe to overlay on the ring region
        xdup_sb = nc.alloc_sbuf_tensor_at("xdup_sb", [27, NPT], F16,
                                          offset=RING_BASE).ap()
        f1_buf = bb.tile([64, G + NPT + G], F16)
        for c in range(4):
            nc.sync.dma_start(xdup_sb[:, c * (NPT // 4):(c + 1) * (NPT // 4)],
                              io["xdup"][:, c * (NPT // 4):(c + 1) * (NPT // 4)])
        nc.vector.memset(f1_buf[:, 0:G], 0.0)
        nc.vector.memset(f1_buf[:, G + NPT:], 0.0)

        for t in range(NPT // 512):
            ps = psb.tile([64, 512], F32, tag="c1")
            nc.tensor.matmul(ps, w1_sb, xdup_sb[:, t * 512:(t + 1) * 512],
                             start=True, stop=True)
            nc.scalar.activation(f1_buf[:, G + t * 512:G + (t + 1) * 512], ps,
                                 AF.Relu, bias=b1_sb)
        _zero_f_borders(nc, f1_buf, 64)

        for t in range(NPT // 512):
            ps = psb.tile([128, 512], F32, tag="c2")
            base = G + t * 512
            for i, d in enumerate(DELTAS):
                nc.tensor.matmul(
                    ps, w2_sb[:, i * 128:(i + 1) * 128],
                    f1_buf[:, base + d:base + d + 512],
                    start=(i == 0), stop=(i == 8))
            nc.scalar.activation(st.f_buf[:, base:base + 512], ps, AF.Relu,
                                 bias=b2_sb)
        _zero_f_borders(nc, st.f_buf, 128)

    # pre-summed f (shared by both capsule layers' iteration-1 preacts);
    # runs on DVE under phase A's conv
    _fsums_compute(nc, st)

    def act16_tile(tag):
        return st.small.tile([64, 16], F16, tag="act16", name=tag)

    route_ctx = ExitStack()
    st.ps_agT = route_ctx.enter_context(
        tc.tile_pool(name="psagt", bufs=1, space="PSUM"))
    st.ps_agr = route_ctx.enter_context(
        tc.tile_pool(name="psagr", bufs=1, space="PSUM"))
    conv_ps_ctx = ExitStack()
    st.ps_votes = conv_ps_ctx.enter_context(
        tc.tile_pool(name="psv", bufs=3, space="PSUM"))
    st.ps_vT = conv_ps_ctx.enter_context(
        tc.tile_pool(name="psvt", bufs=1, space="PSUM"))

    NG = NT // 2

    def run_route_phase(iters, conv=None):
        """Group-granular software pipeline: agree(j) | mid(j-1) |
        preact(j-2), with one-group DMA prefetch. iters: list of
        (si, l, it, pre, mblk, pagt, pagr) route streams (si = stream
        index, per-stream PSUM pools so two streams don't ping-pong one
        bank). conv: optional per-tile hook (phase B interleaves
        layer-1's votes conv)."""
        hold = {}
        sb = [None] * NG     # (agT_sb, g) rows per group
        r8 = [None] * NG     # (r8p, g) rows per group
        for (si, l, *_r) in iters:
            hold[(l, 0)] = _load_group(nc, st, l, 0)
            hold[(l, 1)] = _load_group(nc, st, l, 1)
        for j in range(NG + 4):
            if j < NG:
                if conv is not None:
                    conv(2 * j)
                    conv(2 * j + 1)
                row = []
                for (si, l, it, pre, mblk, pagt, pagr) in iters:
                    if j + 2 < NG:
                        hold[(l, j + 2)] = _load_group(nc, st, l, j + 2)
                    g = hold.pop((l, j))
                    row.append((_agree_mms(nc, st, si, mblk, g, pagt), g))
                sb[j] = row
            if 2 <= j < NG + 2:
                jm = j - 2
                r8[jm] = []
                for (si, l, it, pre, mblk, pagt, pagr), (agT_sb, g) in zip(
                        iters, sb[jm]):
                    r8p = st.r8_pool.tile([128, 128], F8, tag=f"r8p{si}",
                                          name=f"r8p{si}")
                    _route_mid(nc, st, it, jm, agT_sb, r8p, pagr)
                    r8[jm].append((r8p, g))
                sb[jm] = None
            if j >= 4:
                jp = j - 4
                for (si, l, it, pre, mblk, pagt, pagr), (r8p, g) in zip(
                        iters, r8[jp]):
                    _preact_pair(nc, st, pre, r8p, g, jp == NG - 1)
                r8[jp] = None

    # ---- phase A: layer 0 votes conv ----
    wl = load_wl(0)
    for t in range(NT):
        _emit_conv_tile(nc, st, 0, t, wl)
    pre_a = _pre_bank(nc, ps_preA, "preA")
    _iter1_preact(nc, st, pre_a, wl)
    a01 = act16_tile("a01")
    _finish_iter(nc, st, 0, pre_a, None, a01)

    # ---- phase B: layer 1 votes conv + layer 0 iteration 2 ----
    _mblk_build(nc, st, a01, st.mblk[0])
    wl = load_wl(1)
    pre_b0 = _pre_bank(nc, ps_preA, "preA")
    run_route_phase([(0, 0, 2, pre_b0, st.mblk[0], st.ps_agT, st.ps_agr)],
                    conv=lambda t: _emit_conv_tile(nc, st, 1, t, wl))
    conv_ps_ctx.close()
    preB_ctx = ExitStack()
    ps_preB = preB_ctx.enter_context(
        tc.tile_pool(name="pspB", bufs=1, space="PSUM"))
    ps_agT1 = preB_ctx.enter_context(
        tc.tile_pool(name="psagt1", bufs=1, space="PSUM"))
    ps_agr1 = preB_ctx.enter_context(
        tc.tile_pool(name="psagr1", bufs=1, space="PSUM"))
    pre_b1 = _pre_bank(nc, ps_preB, "preB")
    _iter1_preact(nc, st, pre_b1, wl)
    a11 = act16_tile("a11")
    _finish_iter(nc, st, 1, pre_b1, None, a11)
    a02 = act16_tile("a02")
    _finish_iter(nc, st, 0, pre_b0, None, a02)

    # ---- phase C: layer 0 iteration 3 + layer 1 iteration 2 ----
    pre_c0 = _pre_bank(nc, ps_preA, "preA")
    pre_c1 = _pre_bank(nc, ps_preB, "preB")
    _mblk_build(nc, st, a02, st.mblk[0])
    _mblk_build(nc, st, a11, st.mblk[1])
    run_route_phase([
        (0, 0, 3, pre_c0, st.mblk[0], st.ps_agT, st.ps_agr),
        (1, 1, 2, pre_c1, st.mblk[1], ps_agT1, ps_agr1)])
    _finish_iter(nc, st, 0, pre_c0, comb[:, 0:16], None)
    a12 = act16_tile("a12")
    _finish_iter(nc, st, 1, pre_c1, None, a12)
    preB_ctx.close()

    # ---- phase D: layer 1 iteration 3 ----
    pre_d = _pre_bank(nc, ps_preA, "preA")
    _mblk_build(nc, st, a12, st.mblk[0])
    run_route_phase([(0, 1, 3, pre_d, st.mblk[0], st.ps_agT, st.ps_agr)])
    _finish_iter(nc, st, 1, pre_d, comb[:, 16:32], None)

    route_ctx.close()
    caps_ctx.close()

    # ---- SE block ----
    with tc.tile_pool(name="se", bufs=1) as se, \
         tc.tile_pool(name="pse", bufs=1, space="PSUM") as pse:
        ctp = pse.tile([32, 64], F32)
        nc.tensor.transpose(ctp, comb, st.ident[:64, :64])
        ct = se.tile([33, 64], F32)
        nc.vector.memset(ct, 1.0)
        nc.vector.tensor_copy(ct[:32, :], ctp)
        e1p = pse.tile([64, 4], F32)
        nc.tensor.matmul(e1p, ct, se1_sb, start=True, stop=True)
        e1 = se.tile([64, 4], F32)
        nc.scalar.activation(e1, e1p, AF.Relu)
        e1tp = pse.tile([4, 64], F32)
        nc.tensor.transpose(e1tp, e1, st.ident[:64, :64])
        e1t = se.tile([5, 64], F32)
        nc.vector.memset(e1t, 1.0)
        nc.vector.tensor_copy(e1t[:4, :], e1tp)
        e2p = pse.tile([64, 32], F32)
        nc.tensor.matmul(e2p, e1t, se2_sb, start=True, stop=True)
        e2 = se.tile([64, 32], F32)
        nc.scalar.activation(e2, e2p, AF.Sigmoid)
        out_sb = se.tile([64, 32], F32)
        nc.vector.tensor_mul(out_sb, e2, comb)
        nc.sync.dma_start(io["out"], out_sb)


def host_inputs(x, conv1_w, conv1_b, conv2_w, conv2_b, inst_w, inst_b, inst_bias,
                cls_w, cls_b, cls_bias, se_w1, se_b1, se_w2, se_b2):
    """Host-side rearrangement of inputs into the kernel's DRAM layouts."""
    # the capsule-conv bias matmuls are elided on-device: the problem spec
    # fixes inst_b/cls_b to zeros (fill: zeros)
    assert not np.any(inst_b) and not np.any(cls_b), \
        "kernel assumes zero capsule conv biases (problem spec fill=zeros)"
    f4, f2 = np.float32, np.float16
    B = x.shape[0]
    xp = np.zeros((B, 3, PH, PW), f4)
    xp[:, :, 1:-1, 1:-1] = x
    xg = np.zeros((B, 3, 99 + NPT + 99), f4)
    xg[:, :, 99:99 + NP] = xp.reshape(B, 3, NP)
    xdup = np.empty((B, 27, NPT), f2)
    for i, d in enumerate(DELTAS):
        xdup[:, 3 * i:3 * i + 3, :] = xg[:, :, 99 + d:99 + d + NPT]

    w1 = np.ascontiguousarray(conv1_w.transpose(2, 3, 1, 0).reshape(27, 64)).astype(f2)
    w2 = np.ascontiguousarray(
        conv2_w.transpose(1, 2, 3, 0).reshape(64, 9 * 128)).astype(f2)
    wcaps = np.stack([
        np.ascontiguousarray(w.transpose(1, 2, 3, 0).reshape(128, 9, 1024))
        for w in (inst_w, cls_w)]).astype(f2)
    bias2 = np.stack([inst_bias.reshape(64, 16), cls_bias.reshape(64, 16)],
                     axis=1).astype(f4)  # [64, 2, 16]
    se1 = np.concatenate([se_w1.T, se_b1[None, :]], 0).astype(f4)
    se2 = np.concatenate([se_w2.T, se_b2[None, :]], 0).astype(f4)

    shared = {
        "w1": w1, "b1": conv1_b.reshape(64, 1).astype(f4),
        "w2": w2, "b2": conv2_b.reshape(128, 1).astype(f4),
        "wcaps": wcaps, "bias2": bias2,
        "vmask": _border_mask(),
        "dmask": _diag_mask512(),
        "gmask": _gmask(),
        "lspread": _lspread(),
        "se1": se1, "se2": se2,
    }
    return [dict(shared, xdup=np.ascontiguousarray(xdup[b])) for b in range(B)]


_NC_CACHE = None


def _program():
    global _NC_CACHE
    if _NC_CACHE is None:
        _NC_CACHE = build_program()
    return _NC_CACHE


def kernel(**inputs):
    inputs = {k: np.asarray(v, dtype=np.float32) for k, v in inputs.items()}
    in_maps = host_inputs(**inputs)
    nc = _program()
    res = run_bass_kernel_spmd(nc, in_maps, core_ids=list(range(N_CORES)))
    return np.stack([res.results[b]["out"].reshape(8, 8, 32)
                     for b in range(N_CORES)])


# revision 29
# speedup vs baseline: 1.0006x; 1.0006x over previous
"""Trainium2 Bass kernel for DeformCapsNet (conv backbone + 2 capsule layers with
dynamic routing + SE gating). Data-parallel over batch: 1 sample per NeuronCore.

Self-contained: hardcodes all shapes; host-side numpy only rearranges weights /
im2cols the 3-channel input; all FLOPs run on device.

Algorithm notes (validated in numpy against the jax reference):
  - routing logits are constant along the atom axis -> kept as [p, o*c].
  - 3x3 convs = 9 shifted matmuls over a zero-padded 98x98 flat grid (9604
    positions, padded to 9728 = 76 tiles of 128); border positions compute
    garbage votes that are masked to zero before any reduction over p.
  - votes conv emitted in [p, o*c*a] layout (lhsT = shifted f tile, rhs = W);
    fp16 operands, fp32 PSUM. The conv BIAS matmuls are dropped: inst_b/cls_b
    are zeros by the problem spec (fill: zeros); kernel() asserts this.
  - votes ship to HBM in fp8e4m3 as per-tile [v | vT] groups ([128, 2048]
    each, one DMA per tile; vT via 8 PE transposes of the fp8 v). Numpy
    simulation of the full pipeline shows fp8 vote storage & fp8 routes add
    no error over the baseline (9.9e-3 vs 1.01e-2): the error budget is
    dominated by the fp8 activation quantization in the agree path.
  - iteration-1 preact (uniform 1/8 route) = (1/8) sum_p votes: folded into
    9 small matmuls on pre-summed f: sum_p v[p,:] = sum_d (sum_{p in win_d}
    f[p]) @ W_d. The f window sums are separable row/col DVE reductions
    computed ONCE (f is shared by both capsule layers). Replaces 76 per-tile
    preact matmuls per layer with 18.
  - routing iterations 2/3: agree via fp8 matmuls agT[oc,p] = sum_s
    Mblk_s.T @ vT_s with DoubleRow perf mode (slice pairs -> K=256 per
    instruction); preact via fp8 DoubleRow matmuls pairing consecutive
    p-tiles (r8 softmax outputs written as fp8 pair buffers). PSUM fp32.
  - agree back-transpose in fp16 (agT staged to fp16 SBUF).
  - routing phases run a software pipeline (agree(t) | mid(t-2) |
    preact-pair at odd t) so the in-order PE never stalls on the softmax
    chain; phase B interleaves layer-1's conv with layer-0 iteration 2.
  - DMA-written vote-group ring buffers live at manually placed high-SBUF
    addresses (RING_BASE up): the liveness packer's WAR sync for DMA writes
    into reused ranges was observed to be under-counted; same-tensor ring
    reuse is soundly synced. build_program() asserts no pooled allocation
    enters the ring region.
  - measured end-to-end error vs the fp32 reference ~1e-2 (gate 2e-2).
"""

import numpy as np
from contextlib import ExitStack

import concourse.bass as bass
import concourse.bacc as bacc
import concourse.mybir as mybir
import concourse.tile as tile
from concourse.bass_utils import run_bass_kernel_spmd
from concourse.masks import make_identity

N_CORES = 8
PH = PW = 98
NP = PH * PW            # 9604 padded grid positions
NT = 76                 # p-tiles of 128
NPT = NT * 128          # 9728
G = 128                 # guard columns on each side of f buffers
RING_BASE = 139264      # manual high-SBUF region for DMA-written ring buffers
OFFS = [(dy, dx) for dy in (-1, 0, 1) for dx in (-1, 0, 1)]
DELTAS = [dy * PW + dx for dy, dx in OFFS]
F32 = mybir.dt.float32
F16 = mybir.dt.float16
F8 = mybir.dt.float8e4
AF = mybir.ActivationFunctionType
ALU = mybir.AluOpType
DR = mybir.MatmulPerfMode.DoubleRow


def _gmask():
    """[128, 512] f16: 1.0 at (g*16+a, 64s + 8s+g) for the Mblk spread.
    64-wide slices: the DoubleRow lhsT k-tile stride must be 16B-aligned."""
    m = np.zeros((128, 512), dtype=np.float16)
    for g in range(8):
        for a in range(16):
            for s in range(8):
                m[g * 16 + a, 64 * s + 8 * s + g] = 1.0
    return m


def _lspread():
    """[16, 128] f16: L[a, p] = 1 iff p % 16 == a (atom selector for act_big)."""
    m = np.zeros((16, 128), dtype=np.float16)
    for p in range(128):
        m[p % 16, p] = 1.0
    return m


def _diag_mask512():
    """[64, 512] mask: 1.0 at col (oc%32)*16+a for row oc."""
    m = np.zeros((64, 512), dtype=np.float32)
    for oc in range(64):
        m[oc, (oc % 32) * 16:(oc % 32) * 16 + 16] = 1.0
    return m


def _border_mask():
    """1.0 at interior padded-flat positions, 0.0 at borders/tail. [128, NT]."""
    m = np.zeros((PH, PW), dtype=np.float32)
    m[1:-1, 1:-1] = 1.0
    flat = np.zeros(NPT, dtype=np.float32)
    flat[:NP] = m.reshape(-1)
    return flat.reshape(NT, 128).T.copy()  # [p_local, t]


def _zero_f_borders(nc, buf, nparts):
    """memset border cols of an [nparts, G+NPT+G] padded f buffer (interior part)."""
    v = buf[:nparts, :]
    nc.vector.memset(v[:, G:G + PW], 0.0)                       # top row
    nc.vector.memset(v[:, G + NP - PW:G + NPT], 0.0)            # bottom row + tail
    lc = v[:, G:G + NP].rearrange("p (r c) -> p r c", c=PW)
    nc.vector.memset(lc[:, :, 0:1], 0.0)                        # left col
    nc.vector.memset(lc[:, :, PW - 1:PW], 0.0)                  # right col


def build_program():
    nc = bacc.Bacc(
        "TRN2", target_bir_lowering=False, debug=False, num_devices=N_CORES
    )

    def inp(name, shape, dt=F16):
        return nc.dram_tensor(name, shape, dt, kind="ExternalInput").ap()

    io = {
        "xdup": inp("xdup", [27, NPT]),
        "w1": inp("w1", [27, 64]),
        "b1": inp("b1", [64, 1], F32),
        "w2": inp("w2", [64, 9 * 128]),
        "b2": inp("b2", [128, 1], F32),
        "wcaps": inp("wcaps", [2, 128, 9, 1024]),
        "bias2": inp("bias2", [64, 2, 16], F32),
        "vmask": inp("vmask", [128, NT], F32),
        "dmask": inp("dmask", [64, 512], F32),
        "se1": inp("se1", [33, 4], F32),
        "se2": inp("se2", [5, 32], F32),
        "out": nc.dram_tensor("out", [64, 32], F32, kind="ExternalOutput").ap(),
        # per-tile-pair vote groups: [v(t0) | vT(t0) | v(t1) | vT(t1)], fp8
        "votes": nc.dram_tensor("votes_scratch", [2, NT // 2, 128, 4096],
                                F8).ap(),
        "gmask": inp("gmask", [128, 512]),
        "lspread": inp("lspread", [16, 128]),
    }

    with tile.TileContext(nc) as tc, ExitStack() as ctx:
        _body(ctx, tc, io)
    nc.compile()
    for alloc in nc.m.functions[0].allocations:
        for ml in getattr(alloc, "memorylocations", None) or []:
            if (getattr(ml, "allocated", False) and ml.addr is not None
                    and getattr(ml, "type", "") == "SB"
                    and not ml.name.startswith(("vgr", "xdup_sb"))
                    and len(ml.dims) >= 2):
                assert ml.addr + ml.dims[1] <= RING_BASE, \
                    f"pool alloc {ml.name} overlaps ring region"
    return nc


class _State:
    pass


def _finish_iter(nc, st, l, pre_ps, act_out, act16_out):
    """pre_ps [64,512] psum -> diag-extract + bias -> squash -> act [64,16]."""
    pool = st.small
    b2l = st.bias2_sb[:, l * 16:(l + 1) * 16]
    pre = pool.tile([64, 16], F32, tag="pre")
    for h in range(2):
        rows = slice(32 * h, 32 * h + 32)
        masked = pool.tile([32, 512], F32, tag=f"maskd{h}", bufs=1,
                           name=f"maskd{h}")
        nc.vector.tensor_mul(masked, pre_ps[h][rows, :], st.dmask_sb[rows, :])
        nc.vector.reduce_sum(pre[rows, :],
                             masked.rearrange("p (g a) -> p a g", a=16),
                             axis=mybir.AxisListType.X)
    nc.vector.tensor_add(pre, pre, b2l)
    sq = pool.tile([64, 16], F32, tag="sq")
    ssum = pool.tile([64, 1], F32, tag="ssum")
    nc.scalar.activation(sq, pre, AF.Square, accum_out=ssum)
    nrm = pool.tile([64, 1], F32, tag="nrm")
    nc.scalar.activation(nrm, ssum, AF.Sqrt)
    den = pool.tile([64, 1], F32, tag="den")
    nc.vector.tensor_scalar_add(den, ssum, 1.0)
    rec = pool.tile([64, 1], F32, tag="rec")
    nc.vector.reciprocal(rec, den)
    scl = pool.tile([64, 1], F32, tag="scl")
    nc.vector.tensor_mul(scl, nrm, rec)
    if act_out is not None:
        nc.vector.tensor_scalar(act_out, pre, scl, None, op0=ALU.mult)
    if act16_out is not None:
        nc.vector.tensor_scalar(act16_out, pre, scl, None, op0=ALU.mult)


def _pre_bank(nc, pool, tag):
    """Fresh pair of [64,512] preact psum banks (one per 512-col vote half:
    DoubleRow matmul dst must start at partition 0, so the halves can't be
    packed into quadrants of one bank), explicitly zeroed by DVE. All preact
    matmuls then use start=False: they accumulate where has_written bits are
    stale-set (onto the zeros) and overwrite-zero where clear — correct either
    way, and independent of whole-bank bit-clear semantics."""
    pre = []
    for h in range(2):
        p = pool.tile([64, 512], mybir.dt.float32, tag=f"{tag}{h}", bufs=1,
                      name=f"{tag}{h}")
        nc.vector.memset(p, 0.0)
        pre.append(p)
    return pre


def _fsums_compute(nc, st):
    """Pre-summed f for the uniform-route iteration-1 preact:
    fsums[k, d] = (1/8) * sum over the 96x96 window shifted by delta d of
    f[k, .]. Separable: 3 column-window row sums, then 9 row-window sums.
    Writes st.fsrep [128, 9*64] f16 (each delta's column replicated 64x so
    M=32 slices of it can be fed to the PE as the stationary operand)."""
    fi = st.f_buf[:, G:G + NP].rearrange("k (r c) -> k r c", c=PW)
    rs = [st.small.tile([128, PH], F32, tag=f"rs{i}", bufs=1, name=f"rs{i}")
          for i in range(3)]
    for i, dx in enumerate((-1, 0, 1)):
        nc.vector.reduce_sum(rs[i], fi[:, :, 1 + dx:97 + dx],
                             axis=mybir.AxisListType.X)
    fs32 = st.small.tile([128, 9], F32, tag="fs32", bufs=1)
    for d, (dy, dx) in enumerate(OFFS):
        nc.vector.reduce_sum(fs32[:, d:d + 1], rs[dx + 1][:, 1 + dy:97 + dy],
                             axis=mybir.AxisListType.X)
    fs16 = st.small.tile([128, 9], F16, tag="fs16", bufs=1)
    nc.vector.tensor_scalar_mul(fs16, fs32, 0.125)
    nc.vector.tensor_copy(
        st.fsrep.rearrange("p (d m) -> p d m", m=64),
        fs16.unsqueeze(2).broadcast_to((128, 9, 64)))


def _iter1_preact(nc, st, pre_ps, wl):
    """preact_1 = (1/8) sum_p votes, as 18 fsums x W matmuls (fp16, full
    M=64 — every output row carries the same delta sum; the diag extract in
    _finish_iter keeps the valid ones)."""
    for d in range(9):
        for h in range(2):
            nc.tensor.matmul(
                pre_ps[h],
                st.fsrep[:, d * 64:d * 64 + 64],
                wl[:, d * 1024 + 512 * h:d * 1024 + 512 * h + 512],
                start=False, stop=(d == 8), skip_group_check=True)


def _emit_conv_tile(nc, st, l, t, wl):
    """Votes conv for p-tile t of layer l: 18 fp16 matmuls (two [128,512]
    halves in separate PSUM banks), masked fp8 copy, 8 fp8 PE transposes.
    Tiles are assembled in PAIRS into one [128, 4096] group
    [v(t0) | v(t1) | vT slice-major (s, tile, p)] and shipped with a single
    DMA per pair (halves the Sync-queue store traffic; the slice-major vT
    lets the agree matmuls stream both tiles in one instruction)."""
    tt = t % 2
    if tt == 0:
        st.cur_group = st.vvA_pool.tile([128, 4096], F8, tag="vv",
                                        name="group")
    group = st.cur_group
    base = G + t * 128
    vps_h = [st.ps_votes.tile([128, 512], F32, tag="vps", name=f"vps{h}")
             for h in range(2)]
    # delta-outer order: each shifted f-tile stationary operand feeds both
    # output halves back-to-back (the halves accumulate in different PSUM
    # banks so per-half starts are safe)
    for i, d in enumerate(DELTAS):
        for h in range(2):
            nc.tensor.matmul(vps_h[h],
                             st.f_buf[:, base + d:base + d + 128],
                             wl[:, i * 1024 + h * 512:i * 1024 + h * 512 + 512],
                             start=(i == 0), stop=(i == 8))
    # masked fp8 copy: half h as soon as its 9 matmuls finish
    for h in range(2):
        nc.scalar.activation(
            group[:, tt * 1024 + h * 512:tt * 1024 + h * 512 + 512],
            vps_h[h], AF.Copy, scale=st.vmask_sb[:, t:t + 1])
    # fp8 PE transpose requires output element step of 2: write into every
    # other fp8 slot of a double-width PSUM tile, compact on the DVE copy
    vT_ps = st.ps_vT.tile([128, 2048], F8, tag="vtp", bufs=1)
    vT_v = vT_ps.rearrange("p (s x two) -> p s x two", s=8, two=2)
    for s in range(8):
        nc.tensor.transpose(vT_v[:, s, :, 0],
                            group[:, tt * 1024 + s * 128:tt * 1024 + s * 128 + 128],
                            st.ident8)
    dst = group[:, 2048:4096].rearrange("p (s q x) -> p s q x", s=8, q=2)
    nc.vector.tensor_copy(dst[:, :, tt, :], vT_v[:, :, :, 0])
    if tt == 1:
        nc.sync.dma_start(st.votes_d[l, t // 2], group)


def _softmax_r8(nc, st, lsl, r8_dst):
    """logits slice [128,128] (one group = 2 tiles) -> fp8 routes
    (softmax over capsules, both tiles in one op chain)."""
    e_t = st.rt_pool.tile([128, 128], F32, tag="et")
    nc.scalar.activation(e_t, lsl, AF.Exp)
    s_t = st.rt_pool.tile([128, 16], F32, tag="st")
    nc.vector.reduce_sum(s_t, e_t.rearrange("p (o c) -> p o c", c=8),
                         axis=mybir.AxisListType.X)
    rc = st.rt_pool.tile([128, 16], F32, tag="rc")
    nc.vector.reciprocal(rc, s_t)
    nc.vector.tensor_mul(
        r8_dst.rearrange("p (o c) -> p o c", c=8),
        e_t.rearrange("p (o c) -> p o c", c=8),
        rc.unsqueeze(2).broadcast_to((128, 16, 8)))


def _load_group(nc, st, l, j, alternate=False):
    """Load the [v|vT] group j (tiles 2j, 2j+1) of layer l into a ring slot.
    In the route-only phases (alternate=True) the issuing engine alternates
    so the loads spread over two DMA queues (a single queue saturates below
    the per-core HBM peak); in phase B everything stays on the sync queue --
    scalar-issued loads there stall the conv's PSUM->SBUF copy chain, and
    the sync-queue FIFO behind phase A's stores keeps the xdup/ring overlay
    safe."""
    g = st.vg_ring[st.ring_idx % len(st.vg_ring)]
    eng = nc.scalar if (alternate and st.ring_idx % 2) else nc.sync
    st.ring_idx += 1
    eng.dma_start(g, st.votes_d[l, j])
    return g


def _agree_mms(nc, st, si, mblk, g, pagt):
    """4 DoubleRow agree matmuls per GROUP (vT slice pairs x both tiles,
    N=256), then the PSUM->SBUF fp16 copy so the bank frees early and
    backT's input is staged."""
    agT_ps = pagt.tile([64, 256], F32, tag="agt", bufs=1, name="agt")
    mpair = mblk.rearrange("p (s q) -> p s q", q=64)
    for sp in range(4):
        vt = g[:, 2048 + sp * 512:2048 + sp * 512 + 512] \
            .rearrange("p (two n) -> p two n", two=2)
        nc.tensor.matmul(agT_ps, mpair[:, 2 * sp:2 * sp + 2, :], vt,
                         start=(sp == 0), stop=(sp == 3), perf_mode=DR)
    agT_sb = st.agT_pool.tile([64, 256], F16, tag=f"agts{si}", bufs=3,
                              name=f"agts{si}")
    nc.scalar.activation(agT_sb, agT_ps, AF.Copy)
    return agT_sb


def _route_mid(nc, st, it, j, agT_sb, r8_dst, pagr):
    """agree back-transposes (fp16, both tiles of the group) + logits
    update + batched softmax -> fp8 route pair."""
    agr_ps = pagr.tile([128, 128], F16, tag="agrp", bufs=1, name="agrp")
    for tt in range(2):
        nc.tensor.transpose(agr_ps[:, tt * 64:tt * 64 + 64],
                            agT_sb[:, tt * 128:tt * 128 + 128],
                            st.ident16[:64, :64])
    lsl = st.logits[:, j * 128:(j + 1) * 128]
    if it == 2:
        nc.vector.tensor_copy(lsl, agr_ps)
    else:
        nc.vector.tensor_add(lsl, lsl, agr_ps)
    _softmax_r8(nc, st, lsl, r8_dst)


def _preact_pair(nc, st, pre_ps, r8p, g, last):
    """Two DoubleRow matmuls: preact += r8(t0).T@v(t0) + r8(t1).T@v(t1)."""
    rpair = r8p.rearrange("p (two m) -> p two m", two=2)
    vv = g[:, 0:2048].rearrange("p (two x) -> p two x", two=2)
    for h in range(2):
        nc.tensor.matmul(
            pre_ps[h],
            rpair,
            vv[:, :, 512 * h:512 * h + 512],
            start=False, stop=last, skip_group_check=True, perf_mode=DR)


def _mblk_build(nc, st, act16, mblk):
    """Spread act16 [64,16] into the block-diagonal lhsT Mblk [128, 8*72]:
    slice s cols [72s, 72s+64), nonzeros at (g*16+a, slice-col 8s+g).
    Engines can't shift partitions, so the spread goes via PE: actT = act.T,
    act_big = Lspread.T @ actT, Mblk = Gmask * act_big (one DVE op).
    PSUM temporaries ride the existing agr/agT pool slots (tag reuse) to
    stay inside the 8-bank budget."""
    actT_ps = st.ps_agr.tile([16, 64], F16, tag="agrp", name="mbt", bufs=1)
    nc.tensor.transpose(actT_ps, act16, st.ident16[:64, :64])
    actT_sb = st.small.tile([16, 64], F16, tag="mbts")
    nc.vector.tensor_copy(actT_sb, actT_ps)
    big_ps = st.ps_agT.tile([128, 64], F32, tag="agt", name="mbb", bufs=1)
    nc.tensor.matmul(big_ps, st.lspread, actT_sb, start=True, stop=True)
    act_big = st.small.tile([128, 64], F16, tag="mbbig")
    nc.vector.tensor_copy(act_big, big_ps)
    nc.vector.tensor_mul(
        mblk.rearrange("p (s q) -> p s q", q=64),
        st.gmask_sb.rearrange("p (s q) -> p s q", q=64),
        act_big.unsqueeze(1).broadcast_to((128, 8, 64)))


def _body(ctx, tc, io):
    nc = tc.nc
    st = _State()
    # ALL SBUF pools are open for the whole body: no cross-phase address
    # reuse (a reuse-WAR race was observed with scoped pools). Only PSUM
    # pools are phase-scoped (8 banks force reuse).
    persist = ctx.enter_context(tc.tile_pool(name="persist", bufs=1))
    st.small = ctx.enter_context(tc.tile_pool(name="small", bufs=2))
    st.rt_pool = ctx.enter_context(tc.tile_pool(name="rt", bufs=5))
    st.agT_pool = ctx.enter_context(tc.tile_pool(name="agT", bufs=3))
    st.r8_pool = ctx.enter_context(tc.tile_pool(name="r8", bufs=6))
    wl_pool = ctx.enter_context(tc.tile_pool(name="wl", bufs=1))
    fb_pool = ctx.enter_context(tc.tile_pool(name="fb", bufs=1))
    st.vvA_pool = ctx.enter_context(tc.tile_pool(name="vvA", bufs=4))
    st.votes_d = io["votes"]

    caps_ctx = ExitStack()
    ps_preA = caps_ctx.enter_context(
        tc.tile_pool(name="pspA", bufs=1, space="PSUM"))

    # ---- persistent tensors / constants ----
    st.f_buf = fb_pool.tile([128, G + NPT + G], F16)
    st.logits = persist.tile([128, NT * 64], F32)
    w1_sb = persist.tile([27, 64], F16)
    b1_sb = persist.tile([64, 1], F32)
    w2_sb = persist.tile([64, 9 * 128], F16)
    b2_sb = persist.tile([128, 1], F32)
    st.bias2_sb = persist.tile([64, 2 * 16], F32)
    st.vmask_sb = persist.tile([128, NT], F32)
    st.dmask_sb = persist.tile([64, 512], F32)
    se1_sb = persist.tile([33, 4], F32)
    se2_sb = persist.tile([5, 32], F32)
    st.ident = persist.tile([128, 128], F32)
    st.ident16 = persist.tile([128, 128], F16)
    st.ident8 = persist.tile([128, 128], F8)
    st.gmask_sb = persist.tile([128, 512], F16)
    st.lspread = persist.tile([16, 128], F16)
    st.fsrep = persist.tile([128, 9 * 64], F16)
    st.mblk = [persist.tile([128, 512], F8, name=f"mblk{i}") for i in range(2)]
    comb = persist.tile([64, 32], F32)
    # DMA-written ring buffers live at MANUALLY-placed high-SBUF addresses,
    # outside the liveness packer's reach: Tile's WAR sync for DMA writes
    # into packer-reused ranges was observed to be under-counted (race), while
    # same-tensor reuse sync is sound. build_program() asserts pools stay
    # below RING_BASE.
    off = RING_BASE

    def ring_at(name, width, dt):
        nonlocal off
        h = nc.alloc_sbuf_tensor_at(name, [128, width], dt, offset=off)
        off += width * mybir.dt.size(dt)
        return h.ap()

    st.vg_ring = [ring_at(f"vgr{i}", 4096, F8) for i in range(18)]
    st.ring_idx = 0
    assert off <= 212992, f"ring region overflow: {off}"

    def load_wl(l):
        """One resident weight buffer, reloaded per layer (tag-cycled)."""
        wl = wl_pool.tile([128, 9 * 1024], F16, tag="wl", name=f"wl{l}")
        for i in range(9):
            nc.scalar.dma_start(wl[:, i * 1024:(i + 1) * 1024],
                                io["wcaps"][l, :, i])
        return wl

    for name, sb in [("w1", w1_sb), ("b1", b1_sb), ("w2", w2_sb),
                     ("b2", b2_sb)]:
        nc.sync.dma_start(sb, io[name])
    for name, sb in [("vmask", st.vmask_sb), ("dmask", st.dmask_sb),
                     ("se1", se1_sb), ("se2", se2_sb),
                     ("gmask", st.gmask_sb), ("lspread", st.lspread)]:
        nc.scalar.dma_start(sb, io[name])
    nc.scalar.dma_start(st.bias2_sb, io["bias2"].rearrange("p l a -> p (l a)"))
    make_identity(nc, st.ident)
    make_identity(nc, st.ident16)
    nc.vector.tensor_copy(st.ident8, st.ident16)
    nc.vector.memset(st.f_buf[:, 0:G], 0.0)
    nc.vector.memset(st.f_buf[:, G + NPT:], 0.0)

    # ---- backbone ----
    with tc.tile_pool(name="backbone", bufs=1) as bb, \
         tc.tile_pool(name="psb", bufs=2, space="PSUM") as psb:
        # xdup's lifetime (backbone conv1) strictly precedes the ring's
        # first DMA write (phase B route loads, which the Sync queue issues
        # after phase A's stores) -> safe to overlay on the ring region
        xdup_sb = nc.alloc_sbuf_tensor_at("xdup_sb", [27, NPT], F16,
                                          offset=RING_BASE).ap()
        f1_buf = bb.tile([64, G + NPT + G], F16)
        for c in range(4):
            nc.sync.dma_start(xdup_sb[:, c * (NPT // 4):(c + 1) * (NPT // 4)],
                              io["xdup"][:, c * (NPT // 4):(c + 1) * (NPT // 4)])
        nc.vector.memset(f1_buf[:, 0:G], 0.0)
        nc.vector.memset(f1_buf[:, G + NPT:], 0.0)

        for t in range(NPT // 512):
            ps = psb.tile([64, 512], F32, tag="c1")
            nc.tensor.matmul(ps, w1_sb, xdup_sb[:, t * 512:(t + 1) * 512],
                             start=True, stop=True)
            nc.scalar.activation(f1_buf[:, G + t * 512:G + (t + 1) * 512], ps,
                                 AF.Relu, bias=b1_sb)
        _zero_f_borders(nc, f1_buf, 64)

        for t in range(NPT // 512):
            ps = psb.tile([128, 512], F32, tag="c2")
            base = G + t * 512
            for i, d in enumerate(DELTAS):
                nc.tensor.matmul(
                    ps, w2_sb[:, i * 128:(i + 1) * 128],
                    f1_buf[:, base + d:base + d + 512],
                    start=(i == 0), stop=(i == 8))
            nc.scalar.activation(st.f_buf[:, base:base + 512], ps, AF.Relu,
                                 bias=b2_sb)
        _zero_f_borders(nc, st.f_buf, 128)

    # pre-summed f (shared by both capsule layers' iteration-1 preacts);
    # runs on DVE under phase A's conv
    _fsums_compute(nc, st)

    def act16_tile(tag):
        return st.small.tile([64, 16], F16, tag="act16", name=tag)

    route_ctx = ExitStack()
    st.ps_agT = route_ctx.enter_context(
        tc.tile_pool(name="psagt", bufs=1, space="PSUM"))
    st.ps_agr = route_ctx.enter_context(
        tc.tile_pool(name="psagr", bufs=1, space="PSUM"))
    conv_ps_ctx = ExitStack()
    st.ps_votes = conv_ps_ctx.enter_context(
        tc.tile_pool(name="psv", bufs=3, space="PSUM"))
    st.ps_vT = conv_ps_ctx.enter_context(
        tc.tile_pool(name="psvt", bufs=1, space="PSUM"))

    NG = NT // 2

    def run_route_phase(iters, conv=None):
        """Group-granular software pipeline: agree(j) | mid(j-1) |
        preact(j-2), with one-group DMA prefetch. iters: list of
        (si, l, it, pre, mblk, pagt, pagr) route streams (si = stream
        index, per-stream PSUM pools so two streams don't ping-pong one
        bank). conv: optional per-tile hook (phase B interleaves
        layer-1's votes conv)."""
        hold = {}
        sb = [None] * NG     # (agT_sb, g) rows per group
        r8 = [None] * NG     # (r8p, g) rows per group
        alt = conv is None
        for (si, l, *_r) in iters:
            hold[(l, 0)] = _load_group(nc, st, l, 0, alt)
            hold[(l, 1)] = _load_group(nc, st, l, 1, alt)
        for j in range(NG + 4):
            if j < NG:
                if conv is not None:
                    conv(2 * j)
                    conv(2 * j + 1)
                row = []
                for (si, l, it, pre, mblk, pagt, pagr) in iters:
                    if j + 2 < NG:
                        hold[(l, j + 2)] = _load_group(nc, st, l, j + 2, alt)
                    g = hold.pop((l, j))
                    row.append((_agree_mms(nc, st, si, mblk, g, pagt), g))
                sb[j] = row
            if 2 <= j < NG + 2:
                jm = j - 2
                r8[jm] = []
                for (si, l, it, pre, mblk, pagt, pagr), (agT_sb, g) in zip(
                        iters, sb[jm]):
                    r8p = st.r8_pool.tile([128, 128], F8, tag=f"r8p{si}",
                                          name=f"r8p{si}")
                    _route_mid(nc, st, it, jm, agT_sb, r8p, pagr)
                    r8[jm].append((r8p, g))
                sb[jm] = None
            if j >= 4:
                jp = j - 4
                for (si, l, it, pre, mblk, pagt, pagr), (r8p, g) in zip(
                        iters, r8[jp]):
                    _preact_pair(nc, st, pre, r8p, g, jp == NG - 1)
                r8[jp] = None

    # ---- phase A: layer 0 votes conv ----
    wl = load_wl(0)
    for t in range(NT):
        _emit_conv_tile(nc, st, 0, t, wl)
    pre_a = _pre_bank(nc, ps_preA, "preA")
    _iter1_preact(nc, st, pre_a, wl)
    a01 = act16_tile("a01")
    _finish_iter(nc, st, 0, pre_a, None, a01)

    # ---- phase B: layer 1 votes conv + layer 0 iteration 2 ----
    _mblk_build(nc, st, a01, st.mblk[0])
    wl = load_wl(1)
    pre_b0 = _pre_bank(nc, ps_preA, "preA")
    run_route_phase([(0, 0, 2, pre_b0, st.mblk[0], st.ps_agT, st.ps_agr)],
                    conv=lambda t: _emit_conv_tile(nc, st, 1, t, wl))
    conv_ps_ctx.close()
    preB_ctx = ExitStack()
    ps_preB = preB_ctx.enter_context(
        tc.tile_pool(name="pspB", bufs=1, space="PSUM"))
    ps_agT1 = preB_ctx.enter_context(
        tc.tile_pool(name="psagt1", bufs=1, space="PSUM"))
    ps_agr1 = preB_ctx.enter_context(
        tc.tile_pool(name="psagr1", bufs=1, space="PSUM"))
    pre_b1 = _pre_bank(nc, ps_preB, "preB")
    _iter1_preact(nc, st, pre_b1, wl)
    a11 = act16_tile("a11")
    _finish_iter(nc, st, 1, pre_b1, None, a11)
    a02 = act16_tile("a02")
    _finish_iter(nc, st, 0, pre_b0, None, a02)

    # ---- phase C: layer 0 iteration 3 + layer 1 iteration 2 ----
    pre_c0 = _pre_bank(nc, ps_preA, "preA")
    pre_c1 = _pre_bank(nc, ps_preB, "preB")
    _mblk_build(nc, st, a02, st.mblk[0])
    _mblk_build(nc, st, a11, st.mblk[1])
    run_route_phase([
        (0, 0, 3, pre_c0, st.mblk[0], st.ps_agT, st.ps_agr),
        (1, 1, 2, pre_c1, st.mblk[1], ps_agT1, ps_agr1)])
    _finish_iter(nc, st, 0, pre_c0, comb[:, 0:16], None)
    a12 = act16_tile("a12")
    _finish_iter(nc, st, 1, pre_c1, None, a12)
    preB_ctx.close()

    # ---- phase D: layer 1 iteration 3 ----
    pre_d = _pre_bank(nc, ps_preA, "preA")
    _mblk_build(nc, st, a12, st.mblk[0])
    run_route_phase([(0, 1, 3, pre_d, st.mblk[0], st.ps_agT, st.ps_agr)])
    _finish_iter(nc, st, 1, pre_d, comb[:, 16:32], None)

    route_ctx.close()
    caps_ctx.close()

    # ---- SE block ----
    with tc.tile_pool(name="se", bufs=1) as se, \
         tc.tile_pool(name="pse", bufs=1, space="PSUM") as pse:
        ctp = pse.tile([32, 64], F32)
        nc.tensor.transpose(ctp, comb, st.ident[:64, :64])
        ct = se.tile([33, 64], F32)
        nc.vector.memset(ct, 1.0)
        nc.vector.tensor_copy(ct[:32, :], ctp)
        e1p = pse.tile([64, 4], F32)
        nc.tensor.matmul(e1p, ct, se1_sb, start=True, stop=True)
        e1 = se.tile([64, 4], F32)
        nc.scalar.activation(e1, e1p, AF.Relu)
        e1tp = pse.tile([4, 64], F32)
        nc.tensor.transpose(e1tp, e1, st.ident[:64, :64])
        e1t = se.tile([5, 64], F32)
        nc.vector.memset(e1t, 1.0)
        nc.vector.tensor_copy(e1t[:4, :], e1tp)
        e2p = pse.tile([64, 32], F32)
        nc.tensor.matmul(e2p, e1t, se2_sb, start=True, stop=True)
        e2 = se.tile([64, 32], F32)
        nc.scalar.activation(e2, e2p, AF.Sigmoid)
        out_sb = se.tile([64, 32], F32)
        nc.vector.tensor_mul(out_sb, e2, comb)
        nc.sync.dma_start(io["out"], out_sb)


def host_inputs(x, conv1_w, conv1_b, conv2_w, conv2_b, inst_w, inst_b, inst_bias,
                cls_w, cls_b, cls_bias, se_w1, se_b1, se_w2, se_b2):
    """Host-side rearrangement of inputs into the kernel's DRAM layouts."""
    # the capsule-conv bias matmuls are elided on-device: the problem spec
    # fixes inst_b/cls_b to zeros (fill: zeros)
    assert not np.any(inst_b) and not np.any(cls_b), \
        "kernel assumes zero capsule conv biases (problem spec fill=zeros)"
    f4, f2 = np.float32, np.float16
    B = x.shape[0]
    xp = np.zeros((B, 3, PH, PW), f4)
    xp[:, :, 1:-1, 1:-1] = x
    xg = np.zeros((B, 3, 99 + NPT + 99), f4)
    xg[:, :, 99:99 + NP] = xp.reshape(B, 3, NP)
    xdup = np.empty((B, 27, NPT), f2)
    for i, d in enumerate(DELTAS):
        xdup[:, 3 * i:3 * i + 3, :] = xg[:, :, 99 + d:99 + d + NPT]

    w1 = np.ascontiguousarray(conv1_w.transpose(2, 3, 1, 0).reshape(27, 64)).astype(f2)
    w2 = np.ascontiguousarray(
        conv2_w.transpose(1, 2, 3, 0).reshape(64, 9 * 128)).astype(f2)
    wcaps = np.stack([
        np.ascontiguousarray(w.transpose(1, 2, 3, 0).reshape(128, 9, 1024))
        for w in (inst_w, cls_w)]).astype(f2)
    bias2 = np.stack([inst_bias.reshape(64, 16), cls_bias.reshape(64, 16)],
                     axis=1).astype(f4)  # [64, 2, 16]
    se1 = np.concatenate([se_w1.T, se_b1[None, :]], 0).astype(f4)
    se2 = np.concatenate([se_w2.T, se_b2[None, :]], 0).astype(f4)

    shared = {
        "w1": w1, "b1": conv1_b.reshape(64, 1).astype(f4),
        "w2": w2, "b2": conv2_b.reshape(128, 1).astype(f4),
        "wcaps": wcaps, "bias2": bias2,
        "vmask": _border_mask(),
        "dmask": _diag_mask512(),
        "gmask": _gmask(),
        "lspread": _lspread(),
        "se1": se1, "se2": se2,
    }
    return [dict(shared, xdup=np.ascontiguousarray(xdup[b])) for b in range(B)]


_NC_CACHE = None


def _program():
    global _NC_CACHE
    if _NC_CACHE is None:
        _NC_CACHE = build_program()
    return _NC_CACHE


def kernel(**inputs):
    inputs = {k: np.asarray(v, dtype=np.float32) for k, v in inputs.items()}
    in_maps = host_inputs(**inputs)
    nc = _program()
    res = run_bass_kernel_spmd(nc, in_maps, core_ids=list(range(N_CORES)))
    return np.stack([res.results[b]["out"].reshape(8, 8, 32)
                     for b in range(N_CORES)])


# revision 30
# speedup vs baseline: 1.1861x; 1.1853x over previous
"""Trainium2 Bass kernel for DeformCapsNet (conv backbone + 2 capsule layers with
dynamic routing + SE gating). Data-parallel over batch: 1 sample per NeuronCore.

Self-contained: hardcodes all shapes; host-side numpy only rearranges weights /
im2cols the 3-channel input; all FLOPs run on device.

Algorithm notes (validated in numpy against the jax reference):
  - routing logits are constant along the atom axis -> kept as [p, o*c].
  - 3x3 convs = 9 shifted matmuls over a zero-padded 98x98 flat grid (9604
    positions, padded to 9728 = 76 tiles of 128); border positions compute
    garbage votes that are masked to zero before any reduction over p.
  - votes conv emitted in [p, o*c*a] layout (lhsT = shifted f tile, rhs = W);
    fp16 operands, fp32 PSUM. The conv BIAS matmuls are dropped: inst_b/cls_b
    are zeros by the problem spec (fill: zeros); kernel() asserts this.
  - votes ship to HBM in fp8e4m3 as per-tile [v | vT] groups ([128, 2048]
    each, one DMA per tile; vT via 8 PE transposes of the fp8 v). Numpy
    simulation of the full pipeline shows fp8 vote storage & fp8 routes add
    no error over the baseline (9.9e-3 vs 1.01e-2): the error budget is
    dominated by the fp8 activation quantization in the agree path.
  - iteration-1 preact (uniform 1/8 route) = (1/8) sum_p votes: folded into
    9 small matmuls on pre-summed f: sum_p v[p,:] = sum_d (sum_{p in win_d}
    f[p]) @ W_d. The f window sums are separable row/col DVE reductions
    computed ONCE (f is shared by both capsule layers). Replaces 76 per-tile
    preact matmuls per layer with 18.
  - routing iterations 2/3: agree via fp8 matmuls agT[oc,p] = sum_s
    Mblk_s.T @ vT_s with DoubleRow perf mode (slice pairs -> K=256 per
    instruction); preact via fp8 DoubleRow matmuls pairing consecutive
    p-tiles (r8 softmax outputs written as fp8 pair buffers). PSUM fp32.
  - agree back-transpose in fp16 (agT staged to fp16 SBUF).
  - routing phases run a software pipeline (agree(t) | mid(t-2) |
    preact-pair at odd t) so the in-order PE never stalls on the softmax
    chain; phase B interleaves layer-1's conv with layer-0 iteration 2.
  - DMA-written vote-group ring buffers live at manually placed high-SBUF
    addresses (RING_BASE up): the liveness packer's WAR sync for DMA writes
    into reused ranges was observed to be under-counted; same-tensor ring
    reuse is soundly synced. build_program() asserts no pooled allocation
    enters the ring region.
  - measured end-to-end error vs the fp32 reference ~1e-2 (gate 2e-2).
"""

import numpy as np
from contextlib import ExitStack

import concourse.bass as bass
import concourse.bacc as bacc
import concourse.mybir as mybir
import concourse.tile as tile
from concourse.bass_utils import run_bass_kernel_spmd
from concourse.masks import make_identity

N_CORES = 8
PH = PW = 98
NP = PH * PW            # 9604 padded grid positions
NT = 76                 # p-tiles of 128
NPT = NT * 128          # 9728
G = 128                 # guard columns on each side of f buffers
RING_BASE = 139264      # manual high-SBUF region for DMA-written ring buffers
OFFS = [(dy, dx) for dy in (-1, 0, 1) for dx in (-1, 0, 1)]
DELTAS = [dy * PW + dx for dy, dx in OFFS]
F32 = mybir.dt.float32
F16 = mybir.dt.float16
F8 = mybir.dt.float8e4
AF = mybir.ActivationFunctionType
ALU = mybir.AluOpType
DR = mybir.MatmulPerfMode.DoubleRow


def _gmask():
    """[128, 512] f16: 1.0 at (g*16+a, 64s + 8s+g) for the Mblk spread.
    64-wide slices: the DoubleRow lhsT k-tile stride must be 16B-aligned."""
    m = np.zeros((128, 512), dtype=np.float16)
    for g in range(8):
        for a in range(16):
            for s in range(8):
                m[g * 16 + a, 64 * s + 8 * s + g] = 1.0
    return m


def _lspread():
    """[16, 128] f16: L[a, p] = 1 iff p % 16 == a (atom selector for act_big)."""
    m = np.zeros((16, 128), dtype=np.float16)
    for p in range(128):
        m[p % 16, p] = 1.0
    return m


def _diag_mask512():
    """[64, 512] mask: 1.0 at col (oc%32)*16+a for row oc."""
    m = np.zeros((64, 512), dtype=np.float32)
    for oc in range(64):
        m[oc, (oc % 32) * 16:(oc % 32) * 16 + 16] = 1.0
    return m


def _border_mask():
    """1.0 at interior padded-flat positions, 0.0 at borders/tail. [128, NT]."""
    m = np.zeros((PH, PW), dtype=np.float32)
    m[1:-1, 1:-1] = 1.0
    flat = np.zeros(NPT, dtype=np.float32)
    flat[:NP] = m.reshape(-1)
    return flat.reshape(NT, 128).T.copy()  # [p_local, t]


def _zero_f_borders(nc, buf, nparts):
    """memset border cols of an [nparts, G+NPT+G] padded f buffer (interior part)."""
    v = buf[:nparts, :]
    nc.vector.memset(v[:, G:G + PW], 0.0)                       # top row
    nc.vector.memset(v[:, G + NP - PW:G + NPT], 0.0)            # bottom row + tail
    lc = v[:, G:G + NP].rearrange("p (r c) -> p r c", c=PW)
    nc.vector.memset(lc[:, :, 0:1], 0.0)                        # left col
    nc.vector.memset(lc[:, :, PW - 1:PW], 0.0)                  # right col


def build_program():
    nc = bacc.Bacc(
        "TRN2", target_bir_lowering=False, debug=False, num_devices=N_CORES
    )

    def inp(name, shape, dt=F16):
        return nc.dram_tensor(name, shape, dt, kind="ExternalInput").ap()

    io = {
        "xdup": inp("xdup", [27, NPT]),
        "w1": inp("w1", [27, 64]),
        "b1": inp("b1", [64, 1], F32),
        "w2": inp("w2", [64, 9 * 128]),
        "b2": inp("b2", [128, 1], F32),
        "wcaps": inp("wcaps", [2, 128, 9, 1024]),
        "bias2": inp("bias2", [64, 2, 16], F32),
        "vmask": inp("vmask", [128, NT], F32),
        "dmask": inp("dmask", [64, 512], F32),
        "se1": inp("se1", [33, 4], F32),
        "se2": inp("se2", [5, 32], F32),
        "out": nc.dram_tensor("out", [64, 32], F32, kind="ExternalOutput").ap(),
        # per-tile-pair vote groups: [v(t0) | vT(t0) | v(t1) | vT(t1)], fp8
        "votes": nc.dram_tensor("votes_scratch", [2, NT // 2, 128, 4096],
                                F8).ap(),
        "gmask": inp("gmask", [128, 512]),
        "lspread": inp("lspread", [16, 128]),
    }

    with tile.TileContext(nc) as tc, ExitStack() as ctx:
        _body(ctx, tc, io)
    nc.compile()
    for alloc in nc.m.functions[0].allocations:
        for ml in getattr(alloc, "memorylocations", None) or []:
            if (getattr(ml, "allocated", False) and ml.addr is not None
                    and getattr(ml, "type", "") == "SB"
                    and not ml.name.startswith(("vgr", "xdup_sb"))
                    and len(ml.dims) >= 2):
                assert ml.addr + ml.dims[1] <= RING_BASE, \
                    f"pool alloc {ml.name} overlaps ring region"
    return nc


class _State:
    pass


def _finish_iter(nc, st, l, pre_ps, act_out, act16_out):
    """pre_ps [64,512] psum -> diag-extract + bias -> squash -> act [64,16]."""
    pool = st.small
    b2l = st.bias2_sb[:, l * 16:(l + 1) * 16]
    pre = pool.tile([64, 16], F32, tag="pre")
    for h in range(2):
        rows = slice(32 * h, 32 * h + 32)
        masked = pool.tile([32, 512], F32, tag=f"maskd{h}", bufs=1,
                           name=f"maskd{h}")
        nc.vector.tensor_mul(masked, pre_ps[h][rows, :], st.dmask_sb[rows, :])
        nc.vector.reduce_sum(pre[rows, :],
                             masked.rearrange("p (g a) -> p a g", a=16),
                             axis=mybir.AxisListType.X)
    nc.vector.tensor_add(pre, pre, b2l)
    sq = pool.tile([64, 16], F32, tag="sq")
    ssum = pool.tile([64, 1], F32, tag="ssum")
    nc.scalar.activation(sq, pre, AF.Square, accum_out=ssum)
    nrm = pool.tile([64, 1], F32, tag="nrm")
    nc.scalar.activation(nrm, ssum, AF.Sqrt)
    den = pool.tile([64, 1], F32, tag="den")
    nc.vector.tensor_scalar_add(den, ssum, 1.0)
    rec = pool.tile([64, 1], F32, tag="rec")
    nc.vector.reciprocal(rec, den)
    scl = pool.tile([64, 1], F32, tag="scl")
    nc.vector.tensor_mul(scl, nrm, rec)
    if act_out is not None:
        nc.vector.tensor_scalar(act_out, pre, scl, None, op0=ALU.mult)
    if act16_out is not None:
        nc.vector.tensor_scalar(act16_out, pre, scl, None, op0=ALU.mult)


def _pre_bank(nc, pool, tag):
    """Fresh pair of [64,512] preact psum banks (one per 512-col vote half:
    DoubleRow matmul dst must start at partition 0, so the halves can't be
    packed into quadrants of one bank), explicitly zeroed by DVE. All preact
    matmuls then use start=False: they accumulate where has_written bits are
    stale-set (onto the zeros) and overwrite-zero where clear — correct either
    way, and independent of whole-bank bit-clear semantics."""
    pre = []
    for h in range(2):
        p = pool.tile([64, 512], mybir.dt.float32, tag=f"{tag}{h}", bufs=1,
                      name=f"{tag}{h}")
        nc.vector.memset(p, 0.0)
        pre.append(p)
    return pre


def _fsums_compute(nc, st):
    """Pre-summed f for the uniform-route iteration-1 preact:
    fsums[k, d] = (1/8) * sum over the 96x96 window shifted by delta d of
    f[k, .]. Separable: 3 column-window row sums, then 9 row-window sums.
    Writes st.fsrep [128, 9*64] f16 (each delta's column replicated 64x so
    M=32 slices of it can be fed to the PE as the stationary operand)."""
    fi = st.f_buf[:, G:G + NP].rearrange("k (r c) -> k r c", c=PW)
    rs = [st.small.tile([128, PH], F32, tag=f"rs{i}", bufs=1, name=f"rs{i}")
          for i in range(3)]
    for i, dx in enumerate((-1, 0, 1)):
        nc.vector.reduce_sum(rs[i], fi[:, :, 1 + dx:97 + dx],
                             axis=mybir.AxisListType.X)
    fs32 = st.small.tile([128, 9], F32, tag="fs32", bufs=1)
    for d, (dy, dx) in enumerate(OFFS):
        nc.vector.reduce_sum(fs32[:, d:d + 1], rs[dx + 1][:, 1 + dy:97 + dy],
                             axis=mybir.AxisListType.X)
    fs16 = st.small.tile([128, 9], F16, tag="fs16", bufs=1)
    nc.vector.tensor_scalar_mul(fs16, fs32, 0.125)
    nc.vector.tensor_copy(
        st.fsrep.rearrange("p (d m) -> p d m", m=64),
        fs16.unsqueeze(2).broadcast_to((128, 9, 64)))


def _iter1_preact(nc, st, pre_ps, wl):
    """preact_1 = (1/8) sum_p votes, as 18 fsums x W matmuls (fp16, full
    M=64 — every output row carries the same delta sum; the diag extract in
    _finish_iter keeps the valid ones)."""
    for d in range(9):
        for h in range(2):
            nc.tensor.matmul(
                pre_ps[h],
                st.fsrep[:, d * 64:d * 64 + 64],
                wl[:, d * 1024 + 512 * h:d * 1024 + 512 * h + 512],
                start=False, stop=(d == 8), skip_group_check=True)


def _emit_conv_tile(nc, st, l, t, wl):
    """Votes conv for p-tile t of layer l: 18 fp16 matmuls (two [128,512]
    halves in separate PSUM banks), masked fp8 copy, 8 fp8 PE transposes.
    Tiles are assembled in PAIRS into one [128, 4096] group
    [v(t0) | v(t1) | vT slice-major (s, tile, p)] and shipped with a single
    DMA per pair (halves the Sync-queue store traffic; the slice-major vT
    lets the agree matmuls stream both tiles in one instruction)."""
    tt = t % 2
    if tt == 0:
        st.cur_group = st.vvA_pool.tile([128, 4096], F8, tag="vv",
                                        name="group")
    group = st.cur_group
    base = G + t * 128
    vps_h = [st.ps_votes.tile([128, 512], F32, tag="vps", name=f"vps{h}")
             for h in range(2)]
    # delta-outer order: each shifted f-tile stationary operand feeds both
    # output halves back-to-back (the halves accumulate in different PSUM
    # banks so per-half starts are safe)
    for i, d in enumerate(DELTAS):
        for h in range(2):
            nc.tensor.matmul(vps_h[h],
                             st.f_buf[:, base + d:base + d + 128],
                             wl[:, i * 1024 + h * 512:i * 1024 + h * 512 + 512],
                             start=(i == 0), stop=(i == 8))
    # masked fp8 copy: half h as soon as its 9 matmuls finish
    for h in range(2):
        nc.scalar.activation(
            group[:, tt * 1024 + h * 512:tt * 1024 + h * 512 + 512],
            vps_h[h], AF.Copy, scale=st.vmask_sb[:, t:t + 1])
    # fp8 PE transpose requires output element step of 2: write into every
    # other fp8 slot of a double-width PSUM tile, compact on the DVE copy
    vT_ps = st.ps_vT.tile([128, 2048], F8, tag="vtp", bufs=1)
    vT_v = vT_ps.rearrange("p (s x two) -> p s x two", s=8, two=2)
    for s in range(8):
        nc.tensor.transpose(vT_v[:, s, :, 0],
                            group[:, tt * 1024 + s * 128:tt * 1024 + s * 128 + 128],
                            st.ident8)
    dst = group[:, 2048:4096].rearrange("p (s q x) -> p s q x", s=8, q=2)
    nc.vector.tensor_copy(dst[:, :, tt, :], vT_v[:, :, :, 0])
    if tt == 1:
        nc.sync.dma_start(st.votes_d[l, t // 2], group)


def _softmax_r8(nc, st, lsl, r8_dst):
    """logits slice [128,128] (one group = 2 tiles) -> fp8 routes
    (softmax over capsules, both tiles in one op chain)."""
    e_t = st.rt_pool.tile([128, 128], F32, tag="et")
    nc.scalar.activation(e_t, lsl, AF.Exp)
    s_t = st.rt_pool.tile([128, 16], F32, tag="st")
    nc.vector.reduce_sum(s_t, e_t.rearrange("p (o c) -> p o c", c=8),
                         axis=mybir.AxisListType.X)
    rc = st.rt_pool.tile([128, 16], F32, tag="rc")
    nc.vector.reciprocal(rc, s_t)
    nc.vector.tensor_mul(
        r8_dst.rearrange("p (o c) -> p o c", c=8),
        e_t.rearrange("p (o c) -> p o c", c=8),
        rc.unsqueeze(2).broadcast_to((128, 16, 8)))


def _load_group(nc, st, l, j, alternate=False):
    """Load the [v|vT] group j (tiles 2j, 2j+1) of layer l into a ring slot.
    In the route-only phases (alternate=True) the issuing engine alternates
    so the loads spread over two DMA queues (a single queue saturates below
    the per-core HBM peak; gpsimd is otherwise idle, so its queue absorbs
    the WAR waits for free); in phase B everything stays on the sync queue --
    busy-engine-issued loads there stall the conv's copy chain, and
    the sync-queue FIFO behind phase A's stores keeps the xdup/ring overlay
    safe."""
    g = st.vg_ring[st.ring_idx % len(st.vg_ring)]
    eng = nc.gpsimd if (alternate and st.ring_idx % 2) else nc.sync
    st.ring_idx += 1
    eng.dma_start(g, st.votes_d[l, j])
    return g


def _agree_mms(nc, st, si, mblk, g, pagt):
    """4 DoubleRow agree matmuls per GROUP (vT slice pairs x both tiles,
    N=256), then the PSUM->SBUF fp16 copy so the bank frees early and
    backT's input is staged."""
    agT_ps = pagt.tile([64, 256], F32, tag="agt", bufs=1, name="agt")
    mpair = mblk.rearrange("p (s q) -> p s q", q=64)
    for sp in range(4):
        vt = g[:, 2048 + sp * 512:2048 + sp * 512 + 512] \
            .rearrange("p (two n) -> p two n", two=2)
        nc.tensor.matmul(agT_ps, mpair[:, 2 * sp:2 * sp + 2, :], vt,
                         start=(sp == 0), stop=(sp == 3), perf_mode=DR)
    agT_sb = st.agT_pool.tile([64, 256], F16, tag=f"agts{si}", bufs=3,
                              name=f"agts{si}")
    nc.scalar.activation(agT_sb, agT_ps, AF.Copy)
    return agT_sb


def _route_mid(nc, st, it, j, agT_sb, r8_dst, pagr):
    """agree back-transposes (fp16, both tiles of the group) + logits
    update + batched softmax -> fp8 route pair."""
    agr_ps = pagr.tile([128, 128], F16, tag="agrp", bufs=1, name="agrp")
    for tt in range(2):
        nc.tensor.transpose(agr_ps[:, tt * 64:tt * 64 + 64],
                            agT_sb[:, tt * 128:tt * 128 + 128],
                            st.ident16[:64, :64])
    lsl = st.logits[:, j * 128:(j + 1) * 128]
    if it == 2:
        nc.vector.tensor_copy(lsl, agr_ps)
    else:
        nc.vector.tensor_add(lsl, lsl, agr_ps)
    _softmax_r8(nc, st, lsl, r8_dst)


def _preact_pair(nc, st, pre_ps, r8p, g, last):
    """Two DoubleRow matmuls: preact += r8(t0).T@v(t0) + r8(t1).T@v(t1)."""
    rpair = r8p.rearrange("p (two m) -> p two m", two=2)
    vv = g[:, 0:2048].rearrange("p (two x) -> p two x", two=2)
    for h in range(2):
        nc.tensor.matmul(
            pre_ps[h],
            rpair,
            vv[:, :, 512 * h:512 * h + 512],
            start=False, stop=last, skip_group_check=True, perf_mode=DR)


def _mblk_build(nc, st, act16, mblk):
    """Spread act16 [64,16] into the block-diagonal lhsT Mblk [128, 8*72]:
    slice s cols [72s, 72s+64), nonzeros at (g*16+a, slice-col 8s+g).
    Engines can't shift partitions, so the spread goes via PE: actT = act.T,
    act_big = Lspread.T @ actT, Mblk = Gmask * act_big (one DVE op).
    PSUM temporaries ride the existing agr/agT pool slots (tag reuse) to
    stay inside the 8-bank budget."""
    actT_ps = st.ps_agr.tile([16, 64], F16, tag="agrp", name="mbt", bufs=1)
    nc.tensor.transpose(actT_ps, act16, st.ident16[:64, :64])
    actT_sb = st.small.tile([16, 64], F16, tag="mbts")
    nc.vector.tensor_copy(actT_sb, actT_ps)
    big_ps = st.ps_agT.tile([128, 64], F32, tag="agt", name="mbb", bufs=1)
    nc.tensor.matmul(big_ps, st.lspread, actT_sb, start=True, stop=True)
    act_big = st.small.tile([128, 64], F16, tag="mbbig")
    nc.vector.tensor_copy(act_big, big_ps)
    nc.vector.tensor_mul(
        mblk.rearrange("p (s q) -> p s q", q=64),
        st.gmask_sb.rearrange("p (s q) -> p s q", q=64),
        act_big.unsqueeze(1).broadcast_to((128, 8, 64)))


def _body(ctx, tc, io):
    nc = tc.nc
    st = _State()
    # ALL SBUF pools are open for the whole body: no cross-phase address
    # reuse (a reuse-WAR race was observed with scoped pools). Only PSUM
    # pools are phase-scoped (8 banks force reuse).
    persist = ctx.enter_context(tc.tile_pool(name="persist", bufs=1))
    st.small = ctx.enter_context(tc.tile_pool(name="small", bufs=2))
    st.rt_pool = ctx.enter_context(tc.tile_pool(name="rt", bufs=5))
    st.agT_pool = ctx.enter_context(tc.tile_pool(name="agT", bufs=3))
    st.r8_pool = ctx.enter_context(tc.tile_pool(name="r8", bufs=6))
    wl_pool = ctx.enter_context(tc.tile_pool(name="wl", bufs=1))
    fb_pool = ctx.enter_context(tc.tile_pool(name="fb", bufs=1))
    st.vvA_pool = ctx.enter_context(tc.tile_pool(name="vvA", bufs=4))
    st.votes_d = io["votes"]

    caps_ctx = ExitStack()
    ps_preA = caps_ctx.enter_context(
        tc.tile_pool(name="pspA", bufs=1, space="PSUM"))

    # ---- persistent tensors / constants ----
    st.f_buf = fb_pool.tile([128, G + NPT + G], F16)
    st.logits = persist.tile([128, NT * 64], F32)
    w1_sb = persist.tile([27, 64], F16)
    b1_sb = persist.tile([64, 1], F32)
    w2_sb = persist.tile([64, 9 * 128], F16)
    b2_sb = persist.tile([128, 1], F32)
    st.bias2_sb = persist.tile([64, 2 * 16], F32)
    st.vmask_sb = persist.tile([128, NT], F32)
    st.dmask_sb = persist.tile([64, 512], F32)
    se1_sb = persist.tile([33, 4], F32)
    se2_sb = persist.tile([5, 32], F32)
    st.ident = persist.tile([128, 128], F32)
    st.ident16 = persist.tile([128, 128], F16)
    st.ident8 = persist.tile([128, 128], F8)
    st.gmask_sb = persist.tile([128, 512], F16)
    st.lspread = persist.tile([16, 128], F16)
    st.fsrep = persist.tile([128, 9 * 64], F16)
    st.mblk = [persist.tile([128, 512], F8, name=f"mblk{i}") for i in range(2)]
    comb = persist.tile([64, 32], F32)
    # DMA-written ring buffers live at MANUALLY-placed high-SBUF addresses,
    # outside the liveness packer's reach: Tile's WAR sync for DMA writes
    # into packer-reused ranges was observed to be under-counted (race), while
    # same-tensor reuse sync is sound. build_program() asserts pools stay
    # below RING_BASE.
    off = RING_BASE

    def ring_at(name, width, dt):
        nonlocal off
        h = nc.alloc_sbuf_tensor_at(name, [128, width], dt, offset=off)
        off += width * mybir.dt.size(dt)
        return h.ap()

    st.vg_ring = [ring_at(f"vgr{i}", 4096, F8) for i in range(18)]
    st.ring_idx = 0
    assert off <= 212992, f"ring region overflow: {off}"

    def load_wl(l):
        """One resident weight buffer, reloaded per layer (tag-cycled)."""
        wl = wl_pool.tile([128, 9 * 1024], F16, tag="wl", name=f"wl{l}")
        for i in range(9):
            nc.scalar.dma_start(wl[:, i * 1024:(i + 1) * 1024],
                                io["wcaps"][l, :, i])
        return wl

    for name, sb in [("w1", w1_sb), ("b1", b1_sb), ("w2", w2_sb),
                     ("b2", b2_sb)]:
        nc.sync.dma_start(sb, io[name])
    for name, sb in [("vmask", st.vmask_sb), ("dmask", st.dmask_sb),
                     ("se1", se1_sb), ("se2", se2_sb),
                     ("gmask", st.gmask_sb), ("lspread", st.lspread)]:
        nc.scalar.dma_start(sb, io[name])
    nc.scalar.dma_start(st.bias2_sb, io["bias2"].rearrange("p l a -> p (l a)"))
    make_identity(nc, st.ident)
    make_identity(nc, st.ident16)
    nc.vector.tensor_copy(st.ident8, st.ident16)
    nc.vector.memset(st.f_buf[:, 0:G], 0.0)
    nc.vector.memset(st.f_buf[:, G + NPT:], 0.0)

    # ---- backbone ----
    with tc.tile_pool(name="backbone", bufs=1) as bb, \
         tc.tile_pool(name="psb", bufs=2, space="PSUM") as psb:
        # xdup's lifetime (backbone conv1) strictly precedes the ring's
        # first DMA write (phase B route loads, which the Sync queue issues
        # after phase A's stores) -> safe to overlay on the ring region
        xdup_sb = nc.alloc_sbuf_tensor_at("xdup_sb", [27, NPT], F16,
                                          offset=RING_BASE).ap()
        f1_buf = bb.tile([64, G + NPT + G], F16)
        for c in range(4):
            nc.sync.dma_start(xdup_sb[:, c * (NPT // 4):(c + 1) * (NPT // 4)],
                              io["xdup"][:, c * (NPT // 4):(c + 1) * (NPT // 4)])
        nc.vector.memset(f1_buf[:, 0:G], 0.0)
        nc.vector.memset(f1_buf[:, G + NPT:], 0.0)

        for t in range(NPT // 512):
            ps = psb.tile([64, 512], F32, tag="c1")
            nc.tensor.matmul(ps, w1_sb, xdup_sb[:, t * 512:(t + 1) * 512],
                             start=True, stop=True)
            nc.scalar.activation(f1_buf[:, G + t * 512:G + (t + 1) * 512], ps,
                                 AF.Relu, bias=b1_sb)
        _zero_f_borders(nc, f1_buf, 64)

        for t in range(NPT // 512):
            ps = psb.tile([128, 512], F32, tag="c2")
            base = G + t * 512
            for i, d in enumerate(DELTAS):
                nc.tensor.matmul(
                    ps, w2_sb[:, i * 128:(i + 1) * 128],
                    f1_buf[:, base + d:base + d + 512],
                    start=(i == 0), stop=(i == 8))
            nc.scalar.activation(st.f_buf[:, base:base + 512], ps, AF.Relu,
                                 bias=b2_sb)
        _zero_f_borders(nc, st.f_buf, 128)

    # pre-summed f (shared by both capsule layers' iteration-1 preacts);
    # runs on DVE under phase A's conv
    _fsums_compute(nc, st)

    def act16_tile(tag):
        return st.small.tile([64, 16], F16, tag="act16", name=tag)

    route_ctx = ExitStack()
    st.ps_agT = route_ctx.enter_context(
        tc.tile_pool(name="psagt", bufs=1, space="PSUM"))
    st.ps_agr = route_ctx.enter_context(
        tc.tile_pool(name="psagr", bufs=1, space="PSUM"))
    conv_ps_ctx = ExitStack()
    st.ps_votes = conv_ps_ctx.enter_context(
        tc.tile_pool(name="psv", bufs=3, space="PSUM"))
    st.ps_vT = conv_ps_ctx.enter_context(
        tc.tile_pool(name="psvt", bufs=1, space="PSUM"))

    NG = NT // 2

    def run_route_phase(iters, conv=None):
        """Group-granular software pipeline: agree(j) | mid(j-1) |
        preact(j-2), with one-group DMA prefetch. iters: list of
        (si, l, it, pre, mblk, pagt, pagr) route streams (si = stream
        index, per-stream PSUM pools so two streams don't ping-pong one
        bank). conv: optional per-tile hook (phase B interleaves
        layer-1's votes conv)."""
        hold = {}
        sb = [None] * NG     # (agT_sb, g) rows per group
        r8 = [None] * NG     # (r8p, g) rows per group
        alt = conv is None
        for (si, l, *_r) in iters:
            hold[(l, 0)] = _load_group(nc, st, l, 0, alt)
            hold[(l, 1)] = _load_group(nc, st, l, 1, alt)
        for j in range(NG + 4):
            if j < NG:
                if conv is not None:
                    conv(2 * j)
                    conv(2 * j + 1)
                row = []
                for (si, l, it, pre, mblk, pagt, pagr) in iters:
                    if j + 2 < NG:
                        hold[(l, j + 2)] = _load_group(nc, st, l, j + 2, alt)
                    g = hold.pop((l, j))
                    row.append((_agree_mms(nc, st, si, mblk, g, pagt), g))
                sb[j] = row
            if 2 <= j < NG + 2:
                jm = j - 2
                r8[jm] = []
                for (si, l, it, pre, mblk, pagt, pagr), (agT_sb, g) in zip(
                        iters, sb[jm]):
                    r8p = st.r8_pool.tile([128, 128], F8, tag=f"r8p{si}",
                                          name=f"r8p{si}")
                    _route_mid(nc, st, it, jm, agT_sb, r8p, pagr)
                    r8[jm].append((r8p, g))
                sb[jm] = None
            if j >= 4:
                jp = j - 4
                for (si, l, it, pre, mblk, pagt, pagr), (r8p, g) in zip(
                        iters, r8[jp]):
                    _preact_pair(nc, st, pre, r8p, g, jp == NG - 1)
                r8[jp] = None

    # ---- phase A: layer 0 votes conv ----
    wl = load_wl(0)
    for t in range(NT):
        _emit_conv_tile(nc, st, 0, t, wl)
    pre_a = _pre_bank(nc, ps_preA, "preA")
    _iter1_preact(nc, st, pre_a, wl)
    a01 = act16_tile("a01")
    _finish_iter(nc, st, 0, pre_a, None, a01)

    # ---- phase B: layer 1 votes conv + layer 0 iteration 2 ----
    _mblk_build(nc, st, a01, st.mblk[0])
    wl = load_wl(1)
    pre_b0 = _pre_bank(nc, ps_preA, "preA")
    run_route_phase([(0, 0, 2, pre_b0, st.mblk[0], st.ps_agT, st.ps_agr)],
                    conv=lambda t: _emit_conv_tile(nc, st, 1, t, wl))
    conv_ps_ctx.close()
    preB_ctx = ExitStack()
    ps_preB = preB_ctx.enter_context(
        tc.tile_pool(name="pspB", bufs=1, space="PSUM"))
    ps_agT1 = preB_ctx.enter_context(
        tc.tile_pool(name="psagt1", bufs=1, space="PSUM"))
    ps_agr1 = preB_ctx.enter_context(
        tc.tile_pool(name="psagr1", bufs=1, space="PSUM"))
    pre_b1 = _pre_bank(nc, ps_preB, "preB")
    _iter1_preact(nc, st, pre_b1, wl)
    a11 = act16_tile("a11")
    _finish_iter(nc, st, 1, pre_b1, None, a11)
    a02 = act16_tile("a02")
    _finish_iter(nc, st, 0, pre_b0, None, a02)

    # ---- phase C: layer 0 iteration 3 + layer 1 iteration 2 ----
    pre_c0 = _pre_bank(nc, ps_preA, "preA")
    pre_c1 = _pre_bank(nc, ps_preB, "preB")
    _mblk_build(nc, st, a02, st.mblk[0])
    _mblk_build(nc, st, a11, st.mblk[1])
    run_route_phase([
        (0, 0, 3, pre_c0, st.mblk[0], st.ps_agT, st.ps_agr),
        (1, 1, 2, pre_c1, st.mblk[1], ps_agT1, ps_agr1)])
    _finish_iter(nc, st, 0, pre_c0, comb[:, 0:16], None)
    a12 = act16_tile("a12")
    _finish_iter(nc, st, 1, pre_c1, None, a12)
    preB_ctx.close()

    # ---- phase D: layer 1 iteration 3 ----
    pre_d = _pre_bank(nc, ps_preA, "preA")
    _mblk_build(nc, st, a12, st.mblk[0])
    run_route_phase([(0, 1, 3, pre_d, st.mblk[0], st.ps_agT, st.ps_agr)])
    _finish_iter(nc, st, 1, pre_d, comb[:, 16:32], None)

    route_ctx.close()
    caps_ctx.close()

    # ---- SE block ----
    with tc.tile_pool(name="se", bufs=1) as se, \
         tc.tile_pool(name="pse", bufs=1, space="PSUM") as pse:
        ctp = pse.tile([32, 64], F32)
        nc.tensor.transpose(ctp, comb, st.ident[:64, :64])
        ct = se.tile([33, 64], F32)
        nc.vector.memset(ct, 1.0)
        nc.vector.tensor_copy(ct[:32, :], ctp)
        e1p = pse.tile([64, 4], F32)
        nc.tensor.matmul(e1p, ct, se1_sb, start=True, stop=True)
        e1 = se.tile([64, 4], F32)
        nc.scalar.activation(e1, e1p, AF.Relu)
        e1tp = pse.tile([4, 64], F32)
        nc.tensor.transpose(e1tp, e1, st.ident[:64, :64])
        e1t = se.tile([5, 64], F32)
        nc.vector.memset(e1t, 1.0)
        nc.vector.tensor_copy(e1t[:4, :], e1tp)
        e2p = pse.tile([64, 32], F32)
        nc.tensor.matmul(e2p, e1t, se2_sb, start=True, stop=True)
        e2 = se.tile([64, 32], F32)
        nc.scalar.activation(e2, e2p, AF.Sigmoid)
        out_sb = se.tile([64, 32], F32)
        nc.vector.tensor_mul(out_sb, e2, comb)
        nc.sync.dma_start(io["out"], out_sb)


def host_inputs(x, conv1_w, conv1_b, conv2_w, conv2_b, inst_w, inst_b, inst_bias,
                cls_w, cls_b, cls_bias, se_w1, se_b1, se_w2, se_b2):
    """Host-side rearrangement of inputs into the kernel's DRAM layouts."""
    # the capsule-conv bias matmuls are elided on-device: the problem spec
    # fixes inst_b/cls_b to zeros (fill: zeros)
    assert not np.any(inst_b) and not np.any(cls_b), \
        "kernel assumes zero capsule conv biases (problem spec fill=zeros)"
    f4, f2 = np.float32, np.float16
    B = x.shape[0]
    xp = np.zeros((B, 3, PH, PW), f4)
    xp[:, :, 1:-1, 1:-1] = x
    xg = np.zeros((B, 3, 99 + NPT + 99), f4)
    xg[:, :, 99:99 + NP] = xp.reshape(B, 3, NP)
    xdup = np.empty((B, 27, NPT), f2)
    for i, d in enumerate(DELTAS):
        xdup[:, 3 * i:3 * i + 3, :] = xg[:, :, 99 + d:99 + d + NPT]

    w1 = np.ascontiguousarray(conv1_w.transpose(2, 3, 1, 0).reshape(27, 64)).astype(f2)
    w2 = np.ascontiguousarray(
        conv2_w.transpose(1, 2, 3, 0).reshape(64, 9 * 128)).astype(f2)
    wcaps = np.stack([
        np.ascontiguousarray(w.transpose(1, 2, 3, 0).reshape(128, 9, 1024))
        for w in (inst_w, cls_w)]).astype(f2)
    bias2 = np.stack([inst_bias.reshape(64, 16), cls_bias.reshape(64, 16)],
                     axis=1).astype(f4)  # [64, 2, 16]
    se1 = np.concatenate([se_w1.T, se_b1[None, :]], 0).astype(f4)
    se2 = np.concatenate([se_w2.T, se_b2[None, :]], 0).astype(f4)

    shared = {
        "w1": w1, "b1": conv1_b.reshape(64, 1).astype(f4),
        "w2": w2, "b2": conv2_b.reshape(128, 1).astype(f4),
        "wcaps": wcaps, "bias2": bias2,
        "vmask": _border_mask(),
        "dmask": _diag_mask512(),
        "gmask": _gmask(),
        "lspread": _lspread(),
        "se1": se1, "se2": se2,
    }
    return [dict(shared, xdup=np.ascontiguousarray(xdup[b])) for b in range(B)]


_NC_CACHE = None


def _program():
    global _NC_CACHE
    if _NC_CACHE is None:
        _NC_CACHE = build_program()
    return _NC_CACHE


def kernel(**inputs):
    inputs = {k: np.asarray(v, dtype=np.float32) for k, v in inputs.items()}
    in_maps = host_inputs(**inputs)
    nc = _program()
    res = run_bass_kernel_spmd(nc, in_maps, core_ids=list(range(N_CORES)))
    return np.stack([res.results[b]["out"].reshape(8, 8, 32)
                     for b in range(N_CORES)])


# revision 33
# speedup vs baseline: 1.1931x; 1.0059x over previous
"""Trainium2 Bass kernel for DeformCapsNet (conv backbone + 2 capsule layers with
dynamic routing + SE gating). Data-parallel over batch: 1 sample per NeuronCore.

Self-contained: hardcodes all shapes; host-side numpy only rearranges weights /
im2cols the 3-channel input; all FLOPs run on device.

Algorithm notes (validated in numpy against the jax reference):
  - routing logits are constant along the atom axis -> kept as [p, o*c].
  - 3x3 convs = 9 shifted matmuls over a zero-padded 98x98 flat grid (9604
    positions, padded to 9728 = 76 tiles of 128); border positions compute
    garbage votes that are masked to zero before any reduction over p.
  - votes conv emitted in [p, o*c*a] layout (lhsT = shifted f tile, rhs = W);
    fp16 operands, fp32 PSUM. The conv BIAS matmuls are dropped: inst_b/cls_b
    are zeros by the problem spec (fill: zeros); kernel() asserts this.
  - votes ship to HBM in fp8e4m3 as per-tile [v | vT] groups ([128, 2048]
    each, one DMA per tile; vT via 8 PE transposes of the fp8 v). Numpy
    simulation of the full pipeline shows fp8 vote storage & fp8 routes add
    no error over the baseline (9.9e-3 vs 1.01e-2): the error budget is
    dominated by the fp8 activation quantization in the agree path.
  - iteration-1 preact (uniform 1/8 route) = (1/8) sum_p votes: folded into
    9 small matmuls on pre-summed f: sum_p v[p,:] = sum_d (sum_{p in win_d}
    f[p]) @ W_d. The f window sums are separable row/col DVE reductions
    computed ONCE (f is shared by both capsule layers). Replaces 76 per-tile
    preact matmuls per layer with 18.
  - routing iterations 2/3: agree via fp8 matmuls agT[oc,p] = sum_s
    Mblk_s.T @ vT_s with DoubleRow perf mode (slice pairs -> K=256 per
    instruction); preact via fp8 DoubleRow matmuls pairing consecutive
    p-tiles (r8 softmax outputs written as fp8 pair buffers). PSUM fp32.
  - agree back-transpose in fp16 (agT staged to fp16 SBUF).
  - routing phases run a software pipeline (agree(t) | mid(t-2) |
    preact-pair at odd t) so the in-order PE never stalls on the softmax
    chain; phase B interleaves layer-1's conv with layer-0 iteration 2.
  - DMA-written vote-group ring buffers live at manually placed high-SBUF
    addresses (RING_BASE up): the liveness packer's WAR sync for DMA writes
    into reused ranges was observed to be under-counted; same-tensor ring
    reuse is soundly synced. build_program() asserts no pooled allocation
    enters the ring region.
  - measured end-to-end error vs the fp32 reference ~1e-2 (gate 2e-2).
"""

import numpy as np
from contextlib import ExitStack

import concourse.bass as bass
import concourse.bacc as bacc
import concourse.mybir as mybir
import concourse.tile as tile
from concourse.bass_utils import run_bass_kernel_spmd
from concourse.masks import make_identity

N_CORES = 8
PH = PW = 98
NP = PH * PW            # 9604 padded grid positions
NT = 76                 # p-tiles of 128
NPT = NT * 128          # 9728
G = 128                 # guard columns on each side of f buffers
RING_BASE = 139264      # manual high-SBUF region for DMA-written ring buffers
OFFS = [(dy, dx) for dy in (-1, 0, 1) for dx in (-1, 0, 1)]
DELTAS = [dy * PW + dx for dy, dx in OFFS]
F32 = mybir.dt.float32
F16 = mybir.dt.float16
F8 = mybir.dt.float8e4
AF = mybir.ActivationFunctionType
ALU = mybir.AluOpType
DR = mybir.MatmulPerfMode.DoubleRow


def _gmask():
    """[128, 512] f16: 1.0 at (g*16+a, 64s + 8s+g) for the Mblk spread.
    64-wide slices: the DoubleRow lhsT k-tile stride must be 16B-aligned."""
    m = np.zeros((128, 512), dtype=np.float16)
    for g in range(8):
        for a in range(16):
            for s in range(8):
                m[g * 16 + a, 64 * s + 8 * s + g] = 1.0
    return m


def _lspread():
    """[16, 128] f16: L[a, p] = 1 iff p % 16 == a (atom selector for act_big)."""
    m = np.zeros((16, 128), dtype=np.float16)
    for p in range(128):
        m[p % 16, p] = 1.0
    return m


def _diag_mask512():
    """[64, 512] mask: 1.0 at col (oc%32)*16+a for row oc."""
    m = np.zeros((64, 512), dtype=np.float32)
    for oc in range(64):
        m[oc, (oc % 32) * 16:(oc % 32) * 16 + 16] = 1.0
    return m


def _border_mask():
    """1.0 at interior padded-flat positions, 0.0 at borders/tail. [128, NT]."""
    m = np.zeros((PH, PW), dtype=np.float32)
    m[1:-1, 1:-1] = 1.0
    flat = np.zeros(NPT, dtype=np.float32)
    flat[:NP] = m.reshape(-1)
    return flat.reshape(NT, 128).T.copy()  # [p_local, t]


def _zero_f_borders(nc, buf, nparts):
    """memset border cols of an [nparts, G+NPT+G] padded f buffer (interior part)."""
    v = buf[:nparts, :]
    nc.vector.memset(v[:, G:G + PW], 0.0)                       # top row
    nc.vector.memset(v[:, G + NP - PW:G + NPT], 0.0)            # bottom row + tail
    lc = v[:, G:G + NP].rearrange("p (r c) -> p r c", c=PW)
    nc.vector.memset(lc[:, :, 0:1], 0.0)                        # left col
    nc.vector.memset(lc[:, :, PW - 1:PW], 0.0)                  # right col


def build_program():
    nc = bacc.Bacc(
        "TRN2", target_bir_lowering=False, debug=False, num_devices=N_CORES
    )

    def inp(name, shape, dt=F16):
        return nc.dram_tensor(name, shape, dt, kind="ExternalInput").ap()

    io = {
        "xdup": inp("xdup", [27, NPT]),
        "w1": inp("w1", [27, 64]),
        "b1": inp("b1", [64, 1], F32),
        "w2": inp("w2", [64, 9 * 128]),
        "b2": inp("b2", [128, 1], F32),
        "wcaps": inp("wcaps", [2, 128, 9, 1024]),
        "bias2": inp("bias2", [64, 2, 16], F32),
        "vmask": inp("vmask", [128, NT], F32),
        "dmask": inp("dmask", [64, 512], F32),
        "se1": inp("se1", [33, 4], F32),
        "se2": inp("se2", [5, 32], F32),
        "out": nc.dram_tensor("out", [64, 32], F32, kind="ExternalOutput").ap(),
        # per-tile-pair vote groups: [v(t0) | vT(t0) | v(t1) | vT(t1)], fp8
        "votes": nc.dram_tensor("votes_scratch", [2, NT // 2, 128, 4096],
                                F8).ap(),
        "gmask": inp("gmask", [128, 512]),
        "lspread": inp("lspread", [16, 128]),
    }

    with tile.TileContext(nc) as tc, ExitStack() as ctx:
        _body(ctx, tc, io)
    nc.compile()
    for alloc in nc.m.functions[0].allocations:
        for ml in getattr(alloc, "memorylocations", None) or []:
            if (getattr(ml, "allocated", False) and ml.addr is not None
                    and getattr(ml, "type", "") == "SB"
                    and not ml.name.startswith(("vgr", "xdup_sb", "f1_buf"))
                    and len(ml.dims) >= 2):
                assert ml.addr + ml.dims[1] <= RING_BASE, \
                    f"pool alloc {ml.name} overlaps ring region"
    return nc


class _State:
    pass


def _finish_iter(nc, st, l, pre_ps, act_out, act16_out):
    """pre_ps [64,512] psum -> diag-extract + bias -> squash -> act [64,16]."""
    pool = st.small
    b2l = st.bias2_sb[:, l * 16:(l + 1) * 16]
    pre = pool.tile([64, 16], F32, tag="pre")
    for h in range(2):
        rows = slice(32 * h, 32 * h + 32)
        masked = pool.tile([32, 512], F32, tag=f"maskd{h}", bufs=1,
                           name=f"maskd{h}")
        nc.vector.tensor_mul(masked, pre_ps[h][rows, :], st.dmask_sb[rows, :])
        nc.vector.reduce_sum(pre[rows, :],
                             masked.rearrange("p (g a) -> p a g", a=16),
                             axis=mybir.AxisListType.X)
    nc.vector.tensor_add(pre, pre, b2l)
    sq = pool.tile([64, 16], F32, tag="sq")
    ssum = pool.tile([64, 1], F32, tag="ssum")
    nc.scalar.activation(sq, pre, AF.Square, accum_out=ssum)
    nrm = pool.tile([64, 1], F32, tag="nrm")
    nc.scalar.activation(nrm, ssum, AF.Sqrt)
    den = pool.tile([64, 1], F32, tag="den")
    nc.vector.tensor_scalar_add(den, ssum, 1.0)
    rec = pool.tile([64, 1], F32, tag="rec")
    nc.vector.reciprocal(rec, den)
    scl = pool.tile([64, 1], F32, tag="scl")
    nc.vector.tensor_mul(scl, nrm, rec)
    if act_out is not None:
        nc.vector.tensor_scalar(act_out, pre, scl, None, op0=ALU.mult)
    if act16_out is not None:
        nc.vector.tensor_scalar(act16_out, pre, scl, None, op0=ALU.mult)


def _pre_bank(nc, pool, tag):
    """Fresh pair of [64,512] preact psum banks (one per 512-col vote half:
    DoubleRow matmul dst must start at partition 0, so the halves can't be
    packed into quadrants of one bank), explicitly zeroed by DVE. All preact
    matmuls then use start=False: they accumulate where has_written bits are
    stale-set (onto the zeros) and overwrite-zero where clear — correct either
    way, and independent of whole-bank bit-clear semantics."""
    pre = []
    for h in range(2):
        p = pool.tile([64, 512], mybir.dt.float32, tag=f"{tag}{h}", bufs=1,
                      name=f"{tag}{h}")
        nc.vector.memset(p, 0.0)
        pre.append(p)
    return pre


def _fsums_compute(nc, st):
    """Pre-summed f for the uniform-route iteration-1 preact:
    fsums[k, d] = (1/8) * sum over the 96x96 window shifted by delta d of
    f[k, .]. Separable: 3 column-window row sums, then 9 row-window sums.
    Writes st.fsrep [128, 9*64] f16 (each delta's column replicated 64x so
    M=32 slices of it can be fed to the PE as the stationary operand)."""
    fi = st.f_buf[:, G:G + NP].rearrange("k (r c) -> k r c", c=PW)
    rs = [st.small.tile([128, PH], F32, tag=f"rs{i}", bufs=1, name=f"rs{i}")
          for i in range(3)]
    for i, dx in enumerate((-1, 0, 1)):
        nc.vector.reduce_sum(rs[i], fi[:, :, 1 + dx:97 + dx],
                             axis=mybir.AxisListType.X)
    fs32 = st.small.tile([128, 9], F32, tag="fs32", bufs=1)
    for d, (dy, dx) in enumerate(OFFS):
        nc.vector.reduce_sum(fs32[:, d:d + 1], rs[dx + 1][:, 1 + dy:97 + dy],
                             axis=mybir.AxisListType.X)
    fs16 = st.small.tile([128, 9], F16, tag="fs16", bufs=1)
    nc.vector.tensor_scalar_mul(fs16, fs32, 0.125)
    nc.vector.tensor_copy(
        st.fsrep.rearrange("p (d m) -> p d m", m=64),
        fs16.unsqueeze(2).broadcast_to((128, 9, 64)))


def _iter1_preact(nc, st, pre_ps, wl):
    """preact_1 = (1/8) sum_p votes, as 18 fsums x W matmuls (fp16, full
    M=64 — every output row carries the same delta sum; the diag extract in
    _finish_iter keeps the valid ones)."""
    for d in range(9):
        for h in range(2):
            nc.tensor.matmul(
                pre_ps[h],
                st.fsrep[:, d * 64:d * 64 + 64],
                wl[:, d * 1024 + 512 * h:d * 1024 + 512 * h + 512],
                start=False, stop=(d == 8), skip_group_check=True)


def _emit_conv_tile(nc, st, l, t, wl):
    """Votes conv for p-tile t of layer l: 18 fp16 matmuls (two [128,512]
    halves in separate PSUM banks), masked fp8 copy, 8 fp8 PE transposes.
    Tiles are assembled in PAIRS into one [128, 4096] group
    [v(t0) | v(t1) | vT slice-major (s, tile, p)] and shipped with a single
    DMA per pair (halves the Sync-queue store traffic; the slice-major vT
    lets the agree matmuls stream both tiles in one instruction)."""
    tt = t % 2
    if tt == 0:
        st.cur_group = st.vvA_pool.tile([128, 4096], F8, tag="vv",
                                        name="group")
    group = st.cur_group
    base = G + t * 128
    vps_h = [st.ps_votes.tile([128, 512], F32, tag="vps", name=f"vps{h}")
             for h in range(2)]
    # delta-outer order: each shifted f-tile stationary operand feeds both
    # output halves back-to-back (the halves accumulate in different PSUM
    # banks so per-half starts are safe)
    for i, d in enumerate(DELTAS):
        for h in range(2):
            nc.tensor.matmul(vps_h[h],
                             st.f_buf[:, base + d:base + d + 128],
                             wl[:, i * 1024 + h * 512:i * 1024 + h * 512 + 512],
                             start=(i == 0), stop=(i == 8))
    # masked fp8 copy: half h as soon as its 9 matmuls finish
    for h in range(2):
        nc.scalar.activation(
            group[:, tt * 1024 + h * 512:tt * 1024 + h * 512 + 512],
            vps_h[h], AF.Copy, scale=st.vmask_sb[:, t:t + 1])
    # fp8 PE transpose requires output element step of 2: write into every
    # other fp8 slot of a double-width PSUM tile, compact on the DVE copy
    vT_ps = st.ps_vT.tile([128, 2048], F8, tag="vtp", bufs=1)
    vT_v = vT_ps.rearrange("p (s x two) -> p s x two", s=8, two=2)
    for s in range(8):
        nc.tensor.transpose(vT_v[:, s, :, 0],
                            group[:, tt * 1024 + s * 128:tt * 1024 + s * 128 + 128],
                            st.ident8)
    dst = group[:, 2048:4096].rearrange("p (s q x) -> p s q x", s=8, q=2)
    nc.vector.tensor_copy(dst[:, :, tt, :], vT_v[:, :, :, 0])
    if tt == 1:
        nc.sync.dma_start(st.votes_d[l, t // 2], group)


def _softmax_r8(nc, st, lsl, r8_dst):
    """logits slice [128,128] (one group = 2 tiles) -> fp8 routes
    (softmax over capsules, both tiles in one op chain)."""
    e_t = st.rt_pool.tile([128, 128], F32, tag="et")
    nc.scalar.activation(e_t, lsl, AF.Exp)
    s_t = st.rt_pool.tile([128, 16], F32, tag="st")
    nc.vector.reduce_sum(s_t, e_t.rearrange("p (o c) -> p o c", c=8),
                         axis=mybir.AxisListType.X)
    rc = st.rt_pool.tile([128, 16], F32, tag="rc")
    nc.vector.reciprocal(rc, s_t)
    nc.vector.tensor_mul(
        r8_dst.rearrange("p (o c) -> p o c", c=8),
        e_t.rearrange("p (o c) -> p o c", c=8),
        rc.unsqueeze(2).broadcast_to((128, 16, 8)))


def _load_group(nc, st, l, j, alternate=False):
    """Load the [v|vT] group j (tiles 2j, 2j+1) of layer l into a ring slot.
    In the route-only phases (alternate=True) the issuing engine alternates
    so the loads spread over two DMA queues (a single queue saturates below
    the per-core HBM peak; gpsimd is otherwise idle, so its queue absorbs
    the WAR waits for free); in phase B everything stays on the sync queue --
    busy-engine-issued loads there stall the conv's copy chain, and
    the sync-queue FIFO behind phase A's stores keeps the xdup/ring overlay
    safe."""
    g = st.vg_ring[st.ring_idx % len(st.vg_ring)]
    eng = nc.gpsimd if (alternate and st.ring_idx % 2) else nc.sync
    st.ring_idx += 1
    eng.dma_start(g, st.votes_d[l, j])
    return g


def _agree_mms(nc, st, si, mblk, g, pagt):
    """4 DoubleRow agree matmuls per GROUP (vT slice pairs x both tiles,
    N=256), then the PSUM->SBUF fp16 copy so the bank frees early and
    backT's input is staged."""
    agT_ps = pagt.tile([64, 256], F32, tag="agt", bufs=1, name="agt")
    mpair = mblk.rearrange("p (s q) -> p s q", q=64)
    for sp in range(4):
        vt = g[:, 2048 + sp * 512:2048 + sp * 512 + 512] \
            .rearrange("p (two n) -> p two n", two=2)
        nc.tensor.matmul(agT_ps, mpair[:, 2 * sp:2 * sp + 2, :], vt,
                         start=(sp == 0), stop=(sp == 3), perf_mode=DR)
    agT_sb = st.agT_pool.tile([64, 256], F16, tag=f"agts{si}", bufs=3,
                              name=f"agts{si}")
    nc.scalar.activation(agT_sb, agT_ps, AF.Copy)
    return agT_sb


def _route_mid(nc, st, it, j, agT_sb, r8_dst, pagr):
    """agree back-transposes (fp16, both tiles of the group) + logits
    update + batched softmax -> fp8 route pair."""
    agr_ps = pagr.tile([128, 128], F16, tag="agrp", bufs=1, name="agrp")
    for tt in range(2):
        nc.tensor.transpose(agr_ps[:, tt * 64:tt * 64 + 64],
                            agT_sb[:, tt * 128:tt * 128 + 128],
                            st.ident16[:64, :64])
    lsl = st.logits[:, j * 128:(j + 1) * 128]
    if it == 2:
        nc.vector.tensor_copy(lsl, agr_ps)
    else:
        nc.vector.tensor_add(lsl, lsl, agr_ps)
    _softmax_r8(nc, st, lsl, r8_dst)


def _preact_pair(nc, st, pre_ps, r8p, g, last):
    """Two DoubleRow matmuls: preact += r8(t0).T@v(t0) + r8(t1).T@v(t1)."""
    rpair = r8p.rearrange("p (two m) -> p two m", two=2)
    vv = g[:, 0:2048].rearrange("p (two x) -> p two x", two=2)
    for h in range(2):
        nc.tensor.matmul(
            pre_ps[h],
            rpair,
            vv[:, :, 512 * h:512 * h + 512],
            start=False, stop=last, skip_group_check=True, perf_mode=DR)


def _mblk_build(nc, st, act16, mblk):
    """Spread act16 [64,16] into the block-diagonal lhsT Mblk [128, 8*72]:
    slice s cols [72s, 72s+64), nonzeros at (g*16+a, slice-col 8s+g).
    Engines can't shift partitions, so the spread goes via PE: actT = act.T,
    act_big = Lspread.T @ actT, Mblk = Gmask * act_big (one DVE op).
    PSUM temporaries ride the existing agr/agT pool slots (tag reuse) to
    stay inside the 8-bank budget."""
    actT_ps = st.ps_agr.tile([16, 64], F16, tag="agrp", name="mbt", bufs=1)
    nc.tensor.transpose(actT_ps, act16, st.ident16[:64, :64])
    actT_sb = st.small.tile([16, 64], F16, tag="mbts")
    nc.vector.tensor_copy(actT_sb, actT_ps)
    big_ps = st.ps_agT.tile([128, 64], F32, tag="agt", name="mbb", bufs=1)
    nc.tensor.matmul(big_ps, st.lspread, actT_sb, start=True, stop=True)
    act_big = st.small.tile([128, 64], F16, tag="mbbig")
    nc.vector.tensor_copy(act_big, big_ps)
    nc.vector.tensor_mul(
        mblk.rearrange("p (s q) -> p s q", q=64),
        st.gmask_sb.rearrange("p (s q) -> p s q", q=64),
        act_big.unsqueeze(1).broadcast_to((128, 8, 64)))


def _body(ctx, tc, io):
    nc = tc.nc
    st = _State()
    # ALL SBUF pools are open for the whole body: no cross-phase address
    # reuse (a reuse-WAR race was observed with scoped pools). Only PSUM
    # pools are phase-scoped (8 banks force reuse).
    persist = ctx.enter_context(tc.tile_pool(name="persist", bufs=1))
    st.small = ctx.enter_context(tc.tile_pool(name="small", bufs=2))
    st.rt_pool = ctx.enter_context(tc.tile_pool(name="rt", bufs=5))
    st.agT_pool = ctx.enter_context(tc.tile_pool(name="agT", bufs=3))
    st.r8_pool = ctx.enter_context(tc.tile_pool(name="r8", bufs=6))
    wl_pool = ctx.enter_context(tc.tile_pool(name="wl", bufs=2))
    fb_pool = ctx.enter_context(tc.tile_pool(name="fb", bufs=1))
    st.vvA_pool = ctx.enter_context(tc.tile_pool(name="vvA", bufs=4))
    st.votes_d = io["votes"]

    caps_ctx = ExitStack()
    ps_preA = caps_ctx.enter_context(
        tc.tile_pool(name="pspA", bufs=1, space="PSUM"))

    # ---- persistent tensors / constants ----
    st.f_buf = fb_pool.tile([128, G + NPT + G], F16)
    st.logits = persist.tile([128, NT * 64], F32)
    w1_sb = persist.tile([27, 64], F16)
    b1_sb = persist.tile([64, 1], F32)
    w2_sb = persist.tile([64, 9 * 128], F16)
    b2_sb = persist.tile([128, 1], F32)
    st.bias2_sb = persist.tile([64, 2 * 16], F32)
    st.vmask_sb = persist.tile([128, NT], F32)
    st.dmask_sb = persist.tile([64, 512], F32)
    se1_sb = persist.tile([33, 4], F32)
    se2_sb = persist.tile([5, 32], F32)
    st.ident = persist.tile([128, 128], F32)
    st.ident16 = persist.tile([128, 128], F16)
    st.ident8 = persist.tile([128, 128], F8)
    st.gmask_sb = persist.tile([128, 512], F16)
    st.lspread = persist.tile([16, 128], F16)
    st.fsrep = persist.tile([128, 9 * 64], F16)
    st.mblk = [persist.tile([128, 512], F8, name=f"mblk{i}") for i in range(2)]
    comb = persist.tile([64, 32], F32)
    # DMA-written ring buffers live at MANUALLY-placed high-SBUF addresses,
    # outside the liveness packer's reach: Tile's WAR sync for DMA writes
    # into packer-reused ranges was observed to be under-counted (race), while
    # same-tensor reuse sync is sound. build_program() asserts pools stay
    # below RING_BASE.
    off = RING_BASE

    def ring_at(name, width, dt):
        nonlocal off
        h = nc.alloc_sbuf_tensor_at(name, [128, width], dt, offset=off)
        off += width * mybir.dt.size(dt)
        return h.ap()

    st.vg_ring = [ring_at(f"vgr{i}", 4096, F8) for i in range(18)]
    st.ring_idx = 0
    assert off <= 212992, f"ring region overflow: {off}"

    def load_wl(l):
        """One resident weight buffer, reloaded per layer (tag-cycled)."""
        wl = wl_pool.tile([128, 9 * 1024], F16, tag="wl", name=f"wl{l}")
        for i in range(9):
            nc.scalar.dma_start(wl[:, i * 1024:(i + 1) * 1024],
                                io["wcaps"][l, :, i])
        return wl

    for name, sb in [("w1", w1_sb), ("b1", b1_sb), ("w2", w2_sb),
                     ("b2", b2_sb)]:
        nc.sync.dma_start(sb, io[name])
    for name, sb in [("vmask", st.vmask_sb), ("dmask", st.dmask_sb),
                     ("se1", se1_sb), ("se2", se2_sb),
                     ("gmask", st.gmask_sb), ("lspread", st.lspread)]:
        nc.scalar.dma_start(sb, io[name])
    nc.scalar.dma_start(st.bias2_sb, io["bias2"].rearrange("p l a -> p (l a)"))
    make_identity(nc, st.ident)
    make_identity(nc, st.ident16)
    nc.vector.tensor_copy(st.ident8, st.ident16)
    nc.vector.memset(st.f_buf[:, 0:G], 0.0)
    nc.vector.memset(st.f_buf[:, G + NPT:], 0.0)

    wl0 = load_wl(0)
    # ---- backbone ----
    with tc.tile_pool(name="backbone", bufs=1) as bb, \
         tc.tile_pool(name="psb", bufs=2, space="PSUM") as psb:
        # xdup's lifetime (backbone conv1) strictly precedes the ring's
        # first DMA write (phase B route loads, which the Sync queue issues
        # after phase A's stores) -> safe to overlay on the ring region
        xdup_sb = nc.alloc_sbuf_tensor_at("xdup_sb", [27, NPT], F16,
                                          offset=RING_BASE).ap()
        f1_buf = nc.alloc_sbuf_tensor_at("f1_buf", [64, G + NPT + G], F16,
                                         offset=RING_BASE + 2 * NPT).ap()
        for c in range(4):
            nc.sync.dma_start(xdup_sb[:, c * (NPT // 4):(c + 1) * (NPT // 4)],
                              io["xdup"][:, c * (NPT // 4):(c + 1) * (NPT // 4)])
        nc.vector.memset(f1_buf[:, 0:G], 0.0)
        nc.vector.memset(f1_buf[:, G + NPT:], 0.0)

        for t in range(NPT // 512):
            ps = psb.tile([64, 512], F32, tag="c1", bufs=4)
            nc.tensor.matmul(ps, w1_sb, xdup_sb[:, t * 512:(t + 1) * 512],
                             start=True, stop=True)
            dst = f1_buf[:, G + t * 512:G + (t + 1) * 512]
            # alternate the relu drain across scalar and DVE so the PE's
            # one-matmul-per-chunk stream isn't drain-bound on one engine
            if t % 2 == 0:
                nc.vector.tensor_scalar(dst, ps, b1_sb, 0.0,
                                        op0=ALU.add, op1=ALU.max)
            else:
                nc.scalar.activation(dst, ps, AF.Relu, bias=b1_sb)
        _zero_f_borders(nc, f1_buf, 64)

        for t in range(NPT // 512):
            ps = psb.tile([128, 512], F32, tag="c2")
            base = G + t * 512
            for i, d in enumerate(DELTAS):
                nc.tensor.matmul(
                    ps, w2_sb[:, i * 128:(i + 1) * 128],
                    f1_buf[:, base + d:base + d + 512],
                    start=(i == 0), stop=(i == 8))
            nc.scalar.activation(st.f_buf[:, base:base + 512], ps, AF.Relu,
                                 bias=b2_sb)
        _zero_f_borders(nc, st.f_buf, 128)

    # pre-summed f (shared by both capsule layers' iteration-1 preacts);
    # runs on DVE under phase A's conv
    _fsums_compute(nc, st)

    def act16_tile(tag):
        return st.small.tile([64, 16], F16, tag="act16", name=tag)

    route_ctx = ExitStack()
    st.ps_agT = route_ctx.enter_context(
        tc.tile_pool(name="psagt", bufs=1, space="PSUM"))
    st.ps_agr = route_ctx.enter_context(
        tc.tile_pool(name="psagr", bufs=1, space="PSUM"))
    conv_ps_ctx = ExitStack()
    st.ps_votes = conv_ps_ctx.enter_context(
        tc.tile_pool(name="psv", bufs=3, space="PSUM"))
    st.ps_vT = conv_ps_ctx.enter_context(
        tc.tile_pool(name="psvt", bufs=1, space="PSUM"))

    NG = NT // 2

    def run_route_phase(iters, conv=None):
        """Group-granular software pipeline: agree(j) | mid(j-1) |
        preact(j-2), with one-group DMA prefetch. iters: list of
        (si, l, it, pre, mblk, pagt, pagr) route streams (si = stream
        index, per-stream PSUM pools so two streams don't ping-pong one
        bank). conv: optional per-tile hook (phase B interleaves
        layer-1's votes conv)."""
        hold = {}
        sb = [None] * NG     # (agT_sb, g) rows per group
        r8 = [None] * NG     # (r8p, g) rows per group
        alt = conv is None
        for (si, l, *_r) in iters:
            hold[(l, 0)] = _load_group(nc, st, l, 0, alt)
            hold[(l, 1)] = _load_group(nc, st, l, 1, alt)
        for j in range(NG + 4):
            if j < NG:
                if conv is not None:
                    conv(2 * j)
                    conv(2 * j + 1)
                row = []
                for (si, l, it, pre, mblk, pagt, pagr) in iters:
                    if j + 2 < NG:
                        hold[(l, j + 2)] = _load_group(nc, st, l, j + 2, alt)
                    g = hold.pop((l, j))
                    row.append((_agree_mms(nc, st, si, mblk, g, pagt), g))
                sb[j] = row
            if 2 <= j < NG + 2:
                jm = j - 2
                r8[jm] = []
                for (si, l, it, pre, mblk, pagt, pagr), (agT_sb, g) in zip(
                        iters, sb[jm]):
                    r8p = st.r8_pool.tile([128, 128], F8, tag=f"r8p{si}",
                                          name=f"r8p{si}")
                    _route_mid(nc, st, it, jm, agT_sb, r8p, pagr)
                    r8[jm].append((r8p, g))
                sb[jm] = None
            if j >= 4:
                jp = j - 4
                for (si, l, it, pre, mblk, pagt, pagr), (r8p, g) in zip(
                        iters, r8[jp]):
                    _preact_pair(nc, st, pre, r8p, g, jp == NG - 1)
                r8[jp] = None

    # ---- phase A: layer 0 votes conv ----
    for t in range(NT):
        _emit_conv_tile(nc, st, 0, t, wl0)
    # layer 1's first conv tiles are emitted BEFORE layer 0's iteration-1
    # finish chain, so the PE chews them while DVE/scalar run the squash +
    # Mblk build (no PE bubble at the A->B boundary)
    wl1 = load_wl(1)
    for t in range(4):
        _emit_conv_tile(nc, st, 1, t, wl1)
    pre_a = _pre_bank(nc, ps_preA, "preA")
    _iter1_preact(nc, st, pre_a, wl0)
    a01 = act16_tile("a01")
    _finish_iter(nc, st, 0, pre_a, None, a01)

    # ---- phase B: layer 1 votes conv + layer 0 iteration 2 ----
    _mblk_build(nc, st, a01, st.mblk[0])
    pre_b0 = _pre_bank(nc, ps_preA, "preA")
    run_route_phase([(0, 0, 2, pre_b0, st.mblk[0], st.ps_agT, st.ps_agr)],
                    conv=lambda t: _emit_conv_tile(nc, st, 1, t, wl1)
                    if t >= 4 else None)
    conv_ps_ctx.close()
    preB_ctx = ExitStack()
    ps_preB = preB_ctx.enter_context(
        tc.tile_pool(name="pspB", bufs=1, space="PSUM"))
    ps_agT1 = preB_ctx.enter_context(
        tc.tile_pool(name="psagt1", bufs=1, space="PSUM"))
    ps_agr1 = preB_ctx.enter_context(
        tc.tile_pool(name="psagr1", bufs=1, space="PSUM"))
    pre_b1 = _pre_bank(nc, ps_preB, "preB")
    _iter1_preact(nc, st, pre_b1, wl1)
    a11 = act16_tile("a11")
    _finish_iter(nc, st, 1, pre_b1, None, a11)
    a02 = act16_tile("a02")
    _finish_iter(nc, st, 0, pre_b0, None, a02)

    # ---- phase C: layer 0 iteration 3 + layer 1 iteration 2 ----
    pre_c0 = _pre_bank(nc, ps_preA, "preA")
    pre_c1 = _pre_bank(nc, ps_preB, "preB")
    _mblk_build(nc, st, a02, st.mblk[0])
    _mblk_build(nc, st, a11, st.mblk[1])
    run_route_phase([
        (0, 0, 3, pre_c0, st.mblk[0], st.ps_agT, st.ps_agr),
        (1, 1, 2, pre_c1, st.mblk[1], ps_agT1, ps_agr1)])
    _finish_iter(nc, st, 0, pre_c0, comb[:, 0:16], None)
    a12 = act16_tile("a12")
    _finish_iter(nc, st, 1, pre_c1, None, a12)
    preB_ctx.close()

    # ---- phase D: layer 1 iteration 3 ----
    pre_d = _pre_bank(nc, ps_preA, "preA")
    _mblk_build(nc, st, a12, st.mblk[0])
    run_route_phase([(0, 1, 3, pre_d, st.mblk[0], st.ps_agT, st.ps_agr)])
    _finish_iter(nc, st, 1, pre_d, comb[:, 16:32], None)

    route_ctx.close()
    caps_ctx.close()

    # ---- SE block ----
    with tc.tile_pool(name="se", bufs=1) as se, \
         tc.tile_pool(name="pse", bufs=1, space="PSUM") as pse:
        ctp = pse.tile([32, 64], F32)
        nc.tensor.transpose(ctp, comb, st.ident[:64, :64])
        ct = se.tile([33, 64], F32)
        nc.vector.memset(ct, 1.0)
        nc.vector.tensor_copy(ct[:32, :], ctp)
        e1p = pse.tile([64, 4], F32)
        nc.tensor.matmul(e1p, ct, se1_sb, start=True, stop=True)
        e1 = se.tile([64, 4], F32)
        nc.scalar.activation(e1, e1p, AF.Relu)
        e1tp = pse.tile([4, 64], F32)
        nc.tensor.transpose(e1tp, e1, st.ident[:64, :64])
        e1t = se.tile([5, 64], F32)
        nc.vector.memset(e1t, 1.0)
        nc.vector.tensor_copy(e1t[:4, :], e1tp)
        e2p = pse.tile([64, 32], F32)
        nc.tensor.matmul(e2p, e1t, se2_sb, start=True, stop=True)
        e2 = se.tile([64, 32], F32)
        nc.scalar.activation(e2, e2p, AF.Sigmoid)
        out_sb = se.tile([64, 32], F32)
        nc.vector.tensor_mul(out_sb, e2, comb)
        nc.sync.dma_start(io["out"], out_sb)


def host_inputs(x, conv1_w, conv1_b, conv2_w, conv2_b, inst_w, inst_b, inst_bias,
                cls_w, cls_b, cls_bias, se_w1, se_b1, se_w2, se_b2):
    """Host-side rearrangement of inputs into the kernel's DRAM layouts."""
    # the capsule-conv bias matmuls are elided on-device: the problem spec
    # fixes inst_b/cls_b to zeros (fill: zeros)
    assert not np.any(inst_b) and not np.any(cls_b), \
        "kernel assumes zero capsule conv biases (problem spec fill=zeros)"
    f4, f2 = np.float32, np.float16
    B = x.shape[0]
    xp = np.zeros((B, 3, PH, PW), f4)
    xp[:, :, 1:-1, 1:-1] = x
    xg = np.zeros((B, 3, 99 + NPT + 99), f4)
    xg[:, :, 99:99 + NP] = xp.reshape(B, 3, NP)
    xdup = np.empty((B, 27, NPT), f2)
    for i, d in enumerate(DELTAS):
        xdup[:, 3 * i:3 * i + 3, :] = xg[:, :, 99 + d:99 + d + NPT]

    w1 = np.ascontiguousarray(conv1_w.transpose(2, 3, 1, 0).reshape(27, 64)).astype(f2)
    w2 = np.ascontiguousarray(
        conv2_w.transpose(1, 2, 3, 0).reshape(64, 9 * 128)).astype(f2)
    wcaps = np.stack([
        np.ascontiguousarray(w.transpose(1, 2, 3, 0).reshape(128, 9, 1024))
        for w in (inst_w, cls_w)]).astype(f2)
    bias2 = np.stack([inst_bias.reshape(64, 16), cls_bias.reshape(64, 16)],
                     axis=1).astype(f4)  # [64, 2, 16]
    se1 = np.concatenate([se_w1.T, se_b1[None, :]], 0).astype(f4)
    se2 = np.concatenate([se_w2.T, se_b2[None, :]], 0).astype(f4)

    shared = {
        "w1": w1, "b1": conv1_b.reshape(64, 1).astype(f4),
        "w2": w2, "b2": conv2_b.reshape(128, 1).astype(f4),
        "wcaps": wcaps, "bias2": bias2,
        "vmask": _border_mask(),
        "dmask": _diag_mask512(),
        "gmask": _gmask(),
        "lspread": _lspread(),
        "se1": se1, "se2": se2,
    }
    return [dict(shared, xdup=np.ascontiguousarray(xdup[b])) for b in range(B)]


_NC_CACHE = None


def _program():
    global _NC_CACHE
    if _NC_CACHE is None:
        _NC_CACHE = build_program()
    return _NC_CACHE


def kernel(**inputs):
    inputs = {k: np.asarray(v, dtype=np.float32) for k, v in inputs.items()}
    in_maps = host_inputs(**inputs)
    nc = _program()
    res = run_bass_kernel_spmd(nc, in_maps, core_ids=list(range(N_CORES)))
    return np.stack([res.results[b]["out"].reshape(8, 8, 32)
                     for b in range(N_CORES)])
